# revision 1
# baseline (speedup 1.0000x reference)
"""3-layer GCN encoder (GCNConv + LayerNorm + ReLU) on 8 TRN2 NeuronCores.

Strategy (dst-partitioned graph parallel):
  - Nodes are partitioned across the 8 cores (12500 each, padded to 12544).
  - Per layer l: each core computes h_c = x_c @ W_l for its node slice,
    scales rows by dinv (deg^-1/2) to fold the src-side normalization, casts
    to bf16 and AllGathers so every core holds the full scaled table
    hhat [100352, 128] bf16 in DRAM.
  - Edge phase: edges are grouped by (dst_tile, src_bank); each 128-edge
    chunk is gathered from hhat via dma_gather (int16 indices, 4 source
    banks of 25088 rows each) and scatter-added into the dst tile's PSUM
    via a one-hot matmul: lhsT = S (S[e, d] = dst_rel[e] == d, built with
    one broadcast is_equal per 8 chunks), rhs = gathered messages.
  - Per dst tile: out = PSUM * dinv_dst + b, then LayerNorm (+ReLU for
    layers 0/1), then transpose (PE) back into the feature-major x_cT
    buffer for the next layer's matmul. Layer 2 writes the output slice.

kernel(**inputs) takes the FULL inputs and returns the FULL [100000, 128]
float32 output.
"""
import os
import sys

sys.path.insert(0, "/opt/trn_rl_repo")

import numpy as np
import ml_dtypes

N = 100000
D = 128
NCORES = 8
SPLIT = 12500        # real nodes per core
P = 128
TILES = 98           # ceil(12544 / 128)
NPAD = TILES * P     # 12544 padded nodes per core
NG = NCORES * NPAD   # 100352 global padded rows
NBANK = 4
BANKROWS = NG // NBANK  # 25088 (< 32767, int16-addressable)
EPS = 1e-5

GATHER_GROUP = int(os.environ.get("GCN_G", "32"))   # chunks per dma_gather
S_BATCH = int(os.environ.get("GCN_SB", "8"))        # chunks per is_equal
GBUFS = int(os.environ.get("GCN_GBUFS", "8"))
NLAYERS = int(os.environ.get("GCN_LAYERS", "3"))    # debug: fewer layers
ABLATE = int(os.environ.get("GCN_ABLATE", "5"))     # debug: 1=mm 2=+AG 3=+gather 4=+edge-mm 5=full
MAXCH = int(os.environ.get("GCN_MAXCH", "0"))       # debug: cap chunks (0 = all)

f32 = None  # filled after bass import (lazy so host preprocessing is importable)


def _preprocess(x, edge_index):
    """Host-side graph preprocessing. Returns per-core input arrays and the
    shared chunk schedule."""
    ei = np.asarray(edge_index)
    src = np.concatenate([ei[0], np.arange(N)]).astype(np.int64)
    dst = np.concatenate([ei[1], np.arange(N)]).astype(np.int64)
    M = src.shape[0]

    deg = np.bincount(dst, minlength=N).astype(np.float32)
    dinv = np.zeros(N, np.float32)
    nz = deg > 0
    dinv[nz] = 1.0 / np.sqrt(deg[nz])

    # Node permutation: degree-sorted global tiles, round-robin over cores.
    # Balances per-(tile, bank) edge counts across cores and packs similar-
    # degree nodes into the same tile slot to cut chunk padding.
    p_of = np.empty(N, np.int64)
    p_of[np.argsort(-deg, kind="stable")] = np.arange(N)
    gtile = p_of >> 7
    pos_of = p_of & 127
    core_of = gtile % NCORES
    slot_of = gtile // NCORES
    sidx_of = slot_of * P + pos_of          # row within the core's slice
    ghat_of = core_of * NPAD + sidx_of      # row within the gathered table

    core = core_of[dst]
    t = slot_of[dst]
    drel = pos_of[dst]
    g = ghat_of[src]
    b = g // BANKROWS
    srel = (g - b * BANKROWS).astype(np.int64)

    key = (core * TILES + t) * NBANK + b
    order = np.argsort(key, kind="stable")
    key_s = key[order]
    core_s = core[order]
    srel_s = srel[order]
    drel_s = drel[order]

    cnt = np.bincount(key, minlength=NCORES * TILES * NBANK).reshape(
        NCORES, TILES, NBANK
    )
    K = np.ceil(cnt.max(axis=0) / P).astype(np.int64)  # [TILES, NBANK] shared
    Ltb = (K * P).reshape(-1)                          # padded group lengths
    off2 = np.concatenate([[0], np.cumsum(Ltb)[:-1]])  # group offsets (flat t,b)
    TOT = int(Ltb.sum())                               # padded edges per core
    TOTCH = TOT // P

    # rank of each edge within its (core, t, b) group
    first = np.searchsorted(key_s, key_s, side="left")
    rank = np.arange(M) - first
    pos = off2[(key_s % (TILES * NBANK))] + rank

    srcrel_pad = np.zeros((NCORES, TOT), np.int16)
    dstrel_pad = np.full((NCORES, TOT), -1.0, np.float32)
    srcrel_pad[core_s, pos] = srel_s.astype(np.int16)
    dstrel_pad[core_s, pos] = drel_s.astype(np.float32)

    # schedule: chunk j -> (t, b); bank stream position q
    tb_of_chunk = np.repeat(np.arange(TILES * NBANK), K.reshape(-1))
    t_of_chunk = tb_of_chunk // NBANK
    b_of_chunk = tb_of_chunk % NBANK
    q_of_chunk = np.zeros(TOTCH, np.int64)
    Cb = np.zeros(NBANK, np.int64)
    for j in range(TOTCH):
        bb = b_of_chunk[j]
        q_of_chunk[j] = Cb[bb]
        Cb[bb] += 1

    # per-bank idx streams, wrapped int16 layout [128, C_b * 8]
    gidx = []
    chunks_src = srcrel_pad.reshape(NCORES, TOTCH, P)
    for bb in range(NBANK):
        sel = chunks_src[:, b_of_chunk == bb, :].reshape(NCORES, -1)  # [8, C_b*128]
        w = sel.reshape(NCORES, -1, 16).transpose(0, 2, 1)            # [8, 16, C_b*8]
        gidx.append(np.tile(w, (1, 8, 1)).astype(np.int16))           # [8, 128, C_b*8]

    dstrel_in = dstrel_pad.reshape(NCORES, TOTCH, P).transpose(0, 2, 1)  # [8,128,TOTCH]
    dstrel_in = dstrel_in.astype(ml_dtypes.bfloat16)

    # padded per-core x (feature-major) and dinv, permuted
    x = np.asarray(x, dtype=np.float32)
    x_pad = np.zeros((NCORES, NPAD, D), np.float32)
    x_pad[core_of, sidx_of] = x
    xcT = np.ascontiguousarray(x_pad.transpose(0, 2, 1))  # [8, 128, 12544]

    dinv_pad = np.zeros((NCORES, NPAD), np.float32)
    dinv_pad[core_of, sidx_of] = dinv
    dinv_in = np.ascontiguousarray(
        dinv_pad.reshape(NCORES, TILES, P).transpose(0, 2, 1)
    )  # [8, 128, TILES]

    sched = dict(
        K=K, TOTCH=TOTCH, t_of_chunk=t_of_chunk, b_of_chunk=b_of_chunk,
        q_of_chunk=q_of_chunk, Cb=Cb, core_of=core_of, sidx_of=sidx_of,
    )
    return sched, xcT, dinv_in, dstrel_in, gidx


def _build(sched, Cb):
    from concourse import bass, bacc, mybir, tile
    from concourse.masks import make_identity

    f32 = mybir.dt.float32
    bf16 = mybir.dt.bfloat16
    i16 = mybir.dt.int16

    TOTCH = sched["TOTCH"]
    t_of = sched["t_of_chunk"]
    b_of = sched["b_of_chunk"]
    q_of = sched["q_of_chunk"]
    K = sched["K"]

    # first/last chunk flags per tile
    is_first = np.zeros(TOTCH, bool)
    is_last = np.zeros(TOTCH, bool)
    prev_t = -1
    for j in range(TOTCH):
        if t_of[j] != prev_t:
            is_first[j] = True
            if j > 0:
                is_last[j - 1] = True
            prev_t = t_of[j]
    is_last[TOTCH - 1] = True

    nc = bacc.Bacc("TRN2", debug=False, num_devices=NCORES, num_swdge_queues=4)

    xcT_d = nc.dram_tensor("xcT", [P, NPAD], f32, kind="ExternalInput")
    dinv_d = nc.dram_tensor("dinv", [P, TILES], f32, kind="ExternalInput")
    dstrel_d = nc.dram_tensor("dstrel", [P, TOTCH], bf16, kind="ExternalInput")
    gidx_d = [
        nc.dram_tensor(f"gidx{bb}", [P, int(Cb[bb]) * 8], i16, kind="ExternalInput")
        for bb in range(NBANK)
    ]
    w_d = [nc.dram_tensor(f"w{l}", [P, D], f32, kind="ExternalInput") for l in range(3)]
    brep_d = [nc.dram_tensor(f"brep{l}", [P, D], f32, kind="ExternalInput") for l in range(3)]
    grep_d = [nc.dram_tensor(f"grep{l}", [P, D], f32, kind="ExternalInput") for l in range(3)]
    btrep_d = [nc.dram_tensor(f"btrep{l}", [P, D], f32, kind="ExternalInput") for l in range(3)]
    iota_d = nc.dram_tensor("iota", [P, P], bf16, kind="ExternalInput")
    out_d = nc.dram_tensor("out", [NPAD, D], f32, kind="ExternalOutput")

    with tile.TileContext(nc) as tc:
        with (
            tc.tile_pool(name="singles", bufs=1) as singles,
            tc.tile_pool(name="gpool", bufs=GBUFS) as gpool,
            tc.tile_pool(name="spool", bufs=4) as spool,
            tc.tile_pool(name="ipool", bufs=10) as ipool,
            tc.tile_pool(name="hstg", bufs=2) as hstg,
            tc.tile_pool(name="ln", bufs=3) as lnp,
            tc.tile_pool(name="psacc", bufs=4, space="PSUM") as psacc,
            tc.tile_pool(name="psmm", bufs=2, space="PSUM") as psmm,
            tc.tile_pool(name="pstp", bufs=2, space="PSUM") as pstp,
            tc.tile_pool(name="dram", bufs=1, space="DRAM") as dram,
        ):
            # ---- persistent SBUF state ----
            xcT = singles.tile([P, NPAD], f32)
            nc.sync.dma_start(out=xcT[:], in_=xcT_d[:])
            dinv_t = singles.tile([P, TILES], f32)
            nc.sync.dma_start(out=dinv_t[:], in_=dinv_d[:])
            dstrel_t = singles.tile([P, TOTCH], bf16)
            nc.sync.dma_start(out=dstrel_t[:], in_=dstrel_d[:])

            w_t, brep_t, grep_t, btrep_t = [], [], [], []
            for l in range(3):
                wt = singles.tile([P, D], f32, name=f"w{l}")
                nc.sync.dma_start(out=wt[:], in_=w_d[l][:])
                w_t.append(wt)
                bt = singles.tile([P, D], f32, name=f"brep{l}")
                nc.sync.dma_start(out=bt[:], in_=brep_d[l][:])
                brep_t.append(bt)
                gt = singles.tile([P, D], f32, name=f"grep{l}")
                nc.sync.dma_start(out=gt[:], in_=grep_d[l][:])
                grep_t.append(gt)
                btt = singles.tile([P, D], f32, name=f"btrep{l}")
                nc.sync.dma_start(out=btt[:], in_=btrep_d[l][:])
                btrep_t.append(btt)
            idx_t = []
            for bb in range(NBANK):
                it0 = singles.tile([P, int(Cb[bb]) * 8], i16, name=f"idxr{bb}")
                nc.sync.dma_start(out=it0[:], in_=gidx_d[bb][:])
                idx_t.append(it0)
            iota_t = singles.tile([P, P], bf16)
            nc.sync.dma_start(out=iota_t[:], in_=iota_d[:])
            ident = singles.tile([P, P], f32)
            make_identity(nc, ident[:])
            eps_t = singles.tile([P, 1], f32)
            nc.vector.memset(eps_t[:], EPS)

            # DRAM internals for the allgathered table
            agin_d = dram.tile([NPAD, D], bf16)

            j_cap = TOTCH
            if MAXCH:
                j_cap = MAXCH
                while j_cap > 0 and not is_last[j_cap - 1]:
                    j_cap -= 1

            for l in range(NLAYERS):
                hfull_d = dram.tile([NG, D], bf16, addr_space="Shared",
                                    name=f"hfull{l}")
                # ---- phase A: h_c = x_c @ W, scale by dinv, cast bf16 ----
                HB = 8
                hstage = None
                for t in range(TILES):
                    hps = psmm.tile([P, D], f32, space="PSUM", tag="hps")
                    nc.tensor.matmul(
                        out=hps[:],
                        lhsT=xcT[:, t * P:(t + 1) * P],
                        rhs=w_t[l][:],
                        start=True,
                        stop=True,
                    )
                    if t % HB == 0:
                        hstage = hstg.tile([P, HB, D], bf16, tag="hstage",
                                           name=f"hs{l}_{t}")
                    nc.vector.scalar_tensor_tensor(
                        out=hstage[:, t % HB, :], in0=hps[:],
                        scalar=dinv_t[:, t:t + 1],
                        in1=brep_t[l][:],
                        op0=mybir.AluOpType.mult, op1=mybir.AluOpType.bypass,
                    )
                    if t % HB == HB - 1 or t == TILES - 1:
                        t0 = (t // HB) * HB
                        nb_ = t - t0 + 1
                        nc.sync.dma_start(
                            out=agin_d[t0 * P:(t0 + nb_) * P, :].rearrange(
                                "(c p) d -> p c d", p=P),
                            in_=hstage[:, :nb_, :],
                        )

                if ABLATE < 2:
                    continue
                # ---- phase B: AllGather the scaled table ----
                nc.gpsimd.collective_compute(
                    "AllGather",
                    mybir.AluOpType.bypass,
                    replica_groups=[list(range(NCORES))],
                    ins=[agin_d.opt()],
                    outs=[hfull_d.opt()],
                )

                # ---- phase C/D: edge aggregation + LN per dst tile ----
                if ABLATE < 3:
                    continue
                gtiles = {}   # (bank, group) -> (tile, ng)
                gq = 0
                stile = None
                acc = None
                for j in range(j_cap):
                    t, bb, q = int(t_of[j]), int(b_of[j]), int(q_of[j])
                    grp, slot = divmod(q, GATHER_GROUP)
                    gk = (bb, grp)
                    if gk not in gtiles:
                        ng = min(GATHER_GROUP, int(Cb[bb]) - grp * GATHER_GROUP)
                        gt = gpool.tile([P, GATHER_GROUP, P], bf16, tag="gbuf",
                                        name=f"g{l}_{bb}_{grp}")
                        nc.gpsimd.dma_gather(
                            out_ap=gt[:, :ng, :],
                            in_ap=hfull_d[bb * BANKROWS:(bb + 1) * BANKROWS, :],
                            idxs_ap=idx_t[bb][:, grp * GATHER_GROUP * 8:
                                              (grp * GATHER_GROUP + ng) * 8],
                            num_idxs=ng * P,
                            num_idxs_reg=ng * P,
                            elem_size=P,
                            single_packet=False,
                            queue_num=gq % 4,
                        )
                        gq += 1
                        gtiles[gk] = gt
                    if ABLATE < 4:
                        continue
                    if j % S_BATCH == 0:
                        nb = min(S_BATCH, TOTCH - j)
                        stile = spool.tile([P, S_BATCH, P], bf16, tag="s",
                                           name=f"s{l}_{j}")
                        nc.vector.tensor_tensor(
                            out=stile[:, :nb, :],
                            in0=iota_t[:, None, :].to_broadcast([P, nb, P]),
                            in1=dstrel_t[:, j:j + nb].to_broadcast([P, nb, P]),
                            op=mybir.AluOpType.is_equal,
                        )
                    if is_first[j]:
                        acc = psacc.tile([P, D], f32, space="PSUM", tag="acc",
                                         name=f"acc{l}_{t}")
                    nc.tensor.matmul(
                        out=acc[:],
                        lhsT=stile[:, j % S_BATCH, :],
                        rhs=gtiles[gk][:, slot, :],
                        start=bool(is_first[j]),
                        stop=bool(is_last[j]),
                    )
                    if is_last[j] and ABLATE < 5:
                        y0 = lnp.tile([P, D], f32, tag="y")
                        nc.vector.tensor_copy(out=y0[:], in_=acc[:])
                        if l == NLAYERS - 1:
                            nc.sync.dma_start(
                                out=out_d[t * P:(t + 1) * P, :], in_=y0[:])
                        continue
                    if is_last[j]:
                        # conv = acc * dinv_dst + b
                        conv = lnp.tile([P, D], f32, tag="conv")
                        nc.vector.scalar_tensor_tensor(
                            out=conv[:],
                            in0=acc[:],
                            scalar=dinv_t[:, t:t + 1],
                            in1=brep_t[l][:],
                            op0=mybir.AluOpType.mult,
                            op1=mybir.AluOpType.add,
                        )
                        # LayerNorm
                        stats = lnp.tile([P, 6], f32, tag="stats")
                        nc.vector.bn_stats(out=stats[:], in_=conv[:])
                        mv = lnp.tile([P, 2], f32, tag="mv")
                        nc.vector.bn_aggr(out=mv[:], in_=stats[:])
                        rstd = lnp.tile([P, 1], f32, tag="rstd")
                        nc.scalar.activation(
                            out=rstd[:], in_=mv[:, 1:2],
                            func=mybir.ActivationFunctionType.Sqrt,
                            bias=eps_t[:],
                        )
                        nc.vector.reciprocal(out=rstd[:], in_=rstd[:])
                        xn = lnp.tile([P, D], f32, tag="xn")
                        nc.vector.scalar_tensor_tensor(
                            out=xn[:], in0=conv[:], scalar=mv[:, 0:1],
                            in1=rstd[:].to_broadcast([P, D]),
                            op0=mybir.AluOpType.subtract,
                            op1=mybir.AluOpType.mult,
                        )
                        y = lnp.tile([P, D], f32, tag="y")
                        nc.vector.tensor_mul(out=y[:], in0=xn[:], in1=grep_t[l][:])
                        nc.vector.tensor_add(out=y[:], in0=y[:], in1=btrep_t[l][:])
                        if l < NLAYERS - 1:
                            nc.scalar.activation(
                                out=y[:], in_=y[:],
                                func=mybir.ActivationFunctionType.Relu,
                            )
                            tp = pstp.tile([P, P], f32, space="PSUM", tag="tp")
                            nc.tensor.transpose(
                                out=tp[:], in_=y[:], identity=ident[:]
                            )
                            nc.scalar.copy(
                                out=xcT[:, t * P:(t + 1) * P], in_=tp[:]
                            )
                        else:
                            nc.sync.dma_start(
                                out=out_d[t * P:(t + 1) * P, :], in_=y[:]
                            )

    nc.compile()
    return nc


def _ensure_ntff_hook():
    """The agent image's antenv lacks axon_hooks; synthesize it and register
    the ctypes-based NTFF profile hook so trace=True works."""
    import types

    try:
        from antenv.axon_hooks import get_axon_ntff_profile_hook  # noqa: F401
        return
    except ImportError:
        pass
    import antenv

    mod = types.ModuleType("antenv.axon_hooks")
    mod._hook = None

    def set_axon_ntff_profile_hook(h):
        mod._hook = h

    def get_axon_ntff_profile_hook():
        return mod._hook

    mod.set_axon_ntff_profile_hook = set_axon_ntff_profile_hook
    mod.get_axon_ntff_profile_hook = get_axon_ntff_profile_hook
    sys.modules["antenv.axon_hooks"] = mod
    antenv.axon_hooks = mod
    try:
        from trn_agent_boot.trn_boot import _ntff_profile_via_ctypes

        mod._hook = _ntff_profile_via_ctypes("/opt/axon/libaxon_pjrt.so")
    except Exception as e:  # degrade to no tracing
        print("ntff hook setup failed:", e)


def kernel(**inputs) -> np.ndarray:
    x = np.asarray(inputs["x"], np.float32)
    edge_index = np.asarray(inputs["edge_index"])
    Ws = [np.asarray(inputs[f"W{l}"], np.float32) for l in range(3)]
    bs = [np.asarray(inputs[f"b{l}"], np.float32) for l in range(3)]
    gs = [np.asarray(inputs[f"g{l}"], np.float32) for l in range(3)]
    bts = [np.asarray(inputs[f"bt{l}"], np.float32) for l in range(3)]

    sched, xcT, dinv_in, dstrel_in, gidx = _preprocess(x, edge_index)
    nc = _build(sched, sched["Cb"])

    iota = np.broadcast_to(
        np.arange(P, dtype=np.float32), (P, P)
    ).astype(ml_dtypes.bfloat16)

    in_maps = []
    for c in range(NCORES):
        m = dict(
            xcT=np.ascontiguousarray(xcT[c]),
            dinv=np.ascontiguousarray(dinv_in[c]),
            dstrel=np.ascontiguousarray(dstrel_in[c]),
            iota=np.ascontiguousarray(iota),
        )
        for bb in range(NBANK):
            m[f"gidx{bb}"] = np.ascontiguousarray(gidx[bb][c])
        for l in range(3):
            m[f"w{l}"] = Ws[l]
            m[f"brep{l}"] = np.ascontiguousarray(
                np.broadcast_to(bs[l], (P, D)).astype(np.float32))
            m[f"grep{l}"] = np.ascontiguousarray(
                np.broadcast_to(gs[l], (P, D)).astype(np.float32))
            m[f"btrep{l}"] = np.ascontiguousarray(
                np.broadcast_to(bts[l], (P, D)).astype(np.float32))
        in_maps.append(m)

    from concourse.bass_utils import run_bass_kernel_spmd

    trace = bool(int(os.environ.get("GCN_TRACE", "0")))
    if trace:
        _ensure_ntff_hook()
    res = run_bass_kernel_spmd(
        nc, in_maps, core_ids=list(range(NCORES)), trace=trace
    )
    kernel.last_results = res

    out = np.zeros((N, D), np.float32)
    core_of = sched["core_of"]
    sidx_of = sched["sidx_of"]
    for c in range(NCORES):
        mask = core_of == c
        out[mask] = res.results[c]["out"][sidx_of[mask]]
    return out



# revision 3
# speedup vs baseline: 1.0186x; 1.0186x over previous
"""3-layer GCN encoder on 8 TRN2 NeuronCores — v2.

Strategy (dst-partitioned graph parallel, pipelined):
  - Nodes partitioned across 8 cores (12500 each, padded to NPAD=12800,
    TILES=100 tiles of 128).
  - Per layer, each core holds the full scaled source table
    hfull [8*NPAD, 128] bf16, built by 4 quarter-AllGathers (each quarter =
    25 tiles = 3200 rows per core -> bank of 8*3200=25600 rows,
    int16-addressable).
  - Edge phase: edges grouped by (dst_tile, src_bank); 128-edge chunks
    gathered via dma_gather (4 SWDGE queues) and scatter-added into the dst
    tile's PSUM via one-hot matmuls (S built with is_equal in batches).
  - Per-tile tail: LayerNorm directly on the PSUM accumulator (the dst-side
    deg^-1/2 scale and bias b fold away via LN affine invariance when b==0;
    general path emitted if b!=0), then ReLU, then transpose (PE) and the
    NEXT layer's x@W matmul + dinv_src scale + bf16 staging, so phase A of
    layer l+1 is hidden inside layer l's edge phase. Quarter-AllGathers for
    layer l+1 fire as soon as their 25 tiles are staged.
  - Layer 2's tail writes the output slice instead.

kernel(**inputs) takes FULL inputs, returns the FULL [100000, 128] f32 output.
"""
import os
import sys

sys.path.insert(0, "/opt/trn_rl_repo")

import numpy as np
import ml_dtypes

N = 100000
D = 128
NCORES = 8
P = 128
TILES = 100
NPAD = TILES * P          # 12800
QT = 4                    # quarters (AllGather splits)
QTILES = TILES // QT      # 25 tiles per quarter
QROWS = QTILES * P        # 3200 rows per core per quarter
BANKROWS = NCORES * QROWS # 25600 rows per bank (< 32767)
NBANK = QT
EPS = 1e-5

GATHER_GROUP = int(os.environ.get("GCN_G", "32"))   # chunks per dma_gather
S_BATCH = int(os.environ.get("GCN_SB", "16"))       # chunks per is_equal
GBUFS = int(os.environ.get("GCN_GBUFS", "8"))
NLAYERS = 3


def _preprocess(x, edge_index):
    """Host-side graph preprocessing -> per-core arrays + shared schedule."""
    ei = np.asarray(edge_index)
    src = np.concatenate([ei[0], np.arange(N)]).astype(np.int64)
    dst = np.concatenate([ei[1], np.arange(N)]).astype(np.int64)
    M = src.shape[0]

    deg = np.bincount(dst, minlength=N).astype(np.float32)
    dinv = np.zeros(N, np.float32)
    nz = deg > 0
    dinv[nz] = 1.0 / np.sqrt(deg[nz])

    # degree-sorted global tiles, round-robin over cores
    p_of = np.empty(N, np.int64)
    p_of[np.argsort(-deg, kind="stable")] = np.arange(N)
    gtile = p_of >> 7
    slot_of = gtile // NCORES

    # Balance cores within each slot-octet: a node's source-quarter depends
    # only on its slot, so per-slot core reassignment leaves every node's
    # bank invariant while equalizing cnt[core, tile, bank] across cores
    # (shrinking the max-over-cores chunk padding).
    q_src = np.minimum(slot_of // QTILES, QT - 1)
    nvec = np.zeros((N, NBANK), np.int32)     # in-degree by source quarter
    np.add.at(nvec, (dst, q_src[src]), 1)
    tot_in = nvec.sum(axis=1)

    core_of = np.empty(N, np.int64)
    pos_of = np.empty(N, np.int64)
    order_slot = np.argsort(slot_of, kind="stable")
    bounds = np.searchsorted(slot_of[order_slot], np.arange(slot_of.max() + 2))
    for s in range(slot_of.max() + 1):
        nodes = order_slot[bounds[s]:bounds[s + 1]]
        nodes = nodes[np.argsort(-tot_in[nodes], kind="stable")]
        S = np.zeros((NCORES, NBANK), np.int64)
        caps = np.full(NCORES, P, np.int64)
        filled = np.zeros(NCORES, np.int64)
        for i in nodes:
            v = nvec[i]
            cost = (S + v).max(axis=1).astype(np.float64)
            cost[caps == 0] = np.inf
            c = int(np.argmin(cost))
            core_of[i] = c
            pos_of[i] = filled[c]
            filled[c] += 1
            caps[c] -= 1
            S[c] += v
    sidx_of = slot_of * P + pos_of          # row within the core's slice

    # bank = quarter of the source's slice; brel = core*QROWS + row-in-quarter
    q_of_node = sidx_of // QROWS
    brel_of_node = core_of * QROWS + (sidx_of % QROWS)

    core = core_of[dst]
    t = slot_of[dst]
    drel = pos_of[dst]
    b = q_of_node[src]
    srel = brel_of_node[src]

    key = (core * TILES + t) * NBANK + b
    # sort by group, then by source row within group (HBM locality)
    order = np.lexsort((srel, key))
    key_s = key[order]
    core_s = core[order]
    srel_s = srel[order]
    drel_s = drel[order]

    cnt = np.bincount(key, minlength=NCORES * TILES * NBANK).reshape(
        NCORES, TILES, NBANK
    )
    K = np.ceil(cnt.max(axis=0) / P).astype(np.int64)  # [TILES, NBANK]
    Ltb = (K * P).reshape(-1)
    off2 = np.concatenate([[0], np.cumsum(Ltb)[:-1]])
    TOT = int(Ltb.sum())
    TOTCH = TOT // P

    first = np.searchsorted(key_s, key_s, side="left")
    rank = np.arange(M) - first
    pos = off2[(key_s % (TILES * NBANK))] + rank

    srcrel_pad = np.full((NCORES, TOT), -1, np.int64)
    dstrel_pad = np.full((NCORES, TOT), -1.0, np.float32)
    srcrel_pad[core_s, pos] = srel_s
    dstrel_pad[core_s, pos] = drel_s.astype(np.float32)
    # pad slots: duplicate the previous real index (page locality, no garbage)
    for c in range(NCORES):
        row = srcrel_pad[c]
        bad = row < 0
        if bad.any():
            idxs = np.where(~bad, np.arange(TOT), 0)
            np.maximum.accumulate(idxs, out=idxs)
            row[:] = row[idxs]
            row[row < 0] = 0
    srcrel_pad = srcrel_pad.astype(np.int16)

    # chunk schedule: chunk j -> (t, b), bank stream position q
    tb_of_chunk = np.repeat(np.arange(TILES * NBANK), K.reshape(-1))
    t_of_chunk = tb_of_chunk // NBANK
    b_of_chunk = tb_of_chunk % NBANK
    q_of_chunk = np.zeros(TOTCH, np.int64)
    Cb = np.zeros(NBANK, np.int64)
    for j in range(TOTCH):
        bb = b_of_chunk[j]
        q_of_chunk[j] = Cb[bb]
        Cb[bb] += 1

    # per-bank idx streams, wrapped int16 [128, C_b * 8]
    gidx = []
    chunks_src = srcrel_pad.reshape(NCORES, TOTCH, P)
    for bb in range(NBANK):
        sel = chunks_src[:, b_of_chunk == bb, :].reshape(NCORES, -1)
        w = sel.reshape(NCORES, -1, 16).transpose(0, 2, 1)
        gidx.append(np.tile(w, (1, 8, 1)).astype(np.int16))

    dstrel_in = dstrel_pad.reshape(NCORES, TOTCH, P).transpose(0, 2, 1)
    dstrel_in = dstrel_in.astype(ml_dtypes.bfloat16)

    # per-core x feature-major bf16, and per-source dinv
    x = np.asarray(x, dtype=np.float32)
    x_pad = np.zeros((NCORES, NPAD, D), np.float32)
    x_pad[core_of, sidx_of] = x
    xcT = np.ascontiguousarray(x_pad.transpose(0, 2, 1)).astype(
        ml_dtypes.bfloat16
    )  # [8, 128, 12800]

    dinv_pad = np.zeros((NCORES, NPAD), np.float32)
    dinv_pad[core_of, sidx_of] = dinv
    dinv_in = np.ascontiguousarray(
        dinv_pad.reshape(NCORES, TILES, P).transpose(0, 2, 1)
    )  # [8, 128, TILES]

    sched = dict(
        K=K, TOTCH=TOTCH, t_of_chunk=t_of_chunk, b_of_chunk=b_of_chunk,
        q_of_chunk=q_of_chunk, Cb=Cb, core_of=core_of, sidx_of=sidx_of,
    )
    return sched, xcT, dinv_in, dstrel_in, gidx


def _build(sched, Cb, triv):
    """triv: dict(b=[bool]*3, g=[bool]*3, bt=[bool]*3) — which params are
    trivial (b==0, g==1, bt==0), letting ops be elided at trace time."""
    from concourse import bass, bacc, mybir, tile
    from concourse.masks import make_identity

    f32 = mybir.dt.float32
    bf16 = mybir.dt.bfloat16
    i16 = mybir.dt.int16

    TOTCH = sched["TOTCH"]
    t_of = sched["t_of_chunk"]
    b_of = sched["b_of_chunk"]
    q_of = sched["q_of_chunk"]

    is_first = np.zeros(TOTCH, bool)
    is_last = np.zeros(TOTCH, bool)
    prev_t = -1
    for j in range(TOTCH):
        if t_of[j] != prev_t:
            is_first[j] = True
            if j > 0:
                is_last[j - 1] = True
            prev_t = t_of[j]
    is_last[TOTCH - 1] = True
    # tiles with no chunks at all (pad tiles)
    tiles_with_chunks = sorted(set(int(t) for t in t_of))

    nc = bacc.Bacc("TRN2", debug=False, num_devices=NCORES, num_swdge_queues=4)

    xcT_d = nc.dram_tensor("xcT", [P, NPAD], bf16, kind="ExternalInput")
    dinv_d = nc.dram_tensor("dinv", [P, TILES], f32, kind="ExternalInput")
    dstrel_d = nc.dram_tensor("dstrel", [P, TOTCH], bf16, kind="ExternalInput")
    gidx_d = [
        nc.dram_tensor(f"gidx{bb}", [P, int(Cb[bb]) * 8], i16, kind="ExternalInput")
        for bb in range(NBANK)
    ]
    w_d = [nc.dram_tensor(f"w{l}", [P, D], f32, kind="ExternalInput") for l in range(3)]
    brep_d = [nc.dram_tensor(f"brep{l}", [P, D], f32, kind="ExternalInput")
              for l in range(3)]
    grep_d = [nc.dram_tensor(f"grep{l}", [P, D], f32, kind="ExternalInput")
              for l in range(3)]
    btrep_d = [nc.dram_tensor(f"btrep{l}", [P, D], f32, kind="ExternalInput")
               for l in range(3)]
    iota_d = nc.dram_tensor("iota", [P, P], bf16, kind="ExternalInput")
    out_d = nc.dram_tensor("out", [NPAD, D], f32, kind="ExternalOutput")

    with tile.TileContext(nc) as tc:
        with (
            tc.tile_pool(name="singles", bufs=1) as singles,
            tc.tile_pool(name="gpool", bufs=GBUFS) as gpool,
            tc.tile_pool(name="spool", bufs=3) as spool,
            tc.tile_pool(name="hstg", bufs=3) as hstg,
            tc.tile_pool(name="ln", bufs=3) as lnp,
            tc.tile_pool(name="psacc", bufs=4, space="PSUM") as psacc,
            tc.tile_pool(name="psmm", bufs=2, space="PSUM") as psmm,
            tc.tile_pool(name="pstp", bufs=2, space="PSUM") as pstp,
            tc.tile_pool(name="dram", bufs=1, space="DRAM") as dram,
        ):
            # ---- persistent SBUF state ----
            xcT = singles.tile([P, NPAD], bf16)
            nc.sync.dma_start(out=xcT[:], in_=xcT_d[:])
            dinv_t = singles.tile([P, TILES], f32)
            nc.sync.dma_start(out=dinv_t[:], in_=dinv_d[:])
            dstrel_t = singles.tile([P, TOTCH], bf16)
            nc.sync.dma_start(out=dstrel_t[:], in_=dstrel_d[:])

            w_t, brep_t, grep_t, btrep_t = [], [], [], []
            for l in range(3):
                wt = singles.tile([P, D], f32, name=f"w{l}")
                nc.sync.dma_start(out=wt[:], in_=w_d[l][:])
                w_t.append(wt)
                if not triv["b"][l]:
                    bt = singles.tile([P, D], f32, name=f"brep{l}")
                    nc.sync.dma_start(out=bt[:], in_=brep_d[l][:])
                    brep_t.append(bt)
                else:
                    brep_t.append(None)
                if not triv["g"][l]:
                    gt = singles.tile([P, D], f32, name=f"grep{l}")
                    nc.sync.dma_start(out=gt[:], in_=grep_d[l][:])
                    grep_t.append(gt)
                else:
                    grep_t.append(None)
                if not triv["bt"][l]:
                    btt = singles.tile([P, D], f32, name=f"btrep{l}")
                    nc.sync.dma_start(out=btt[:], in_=btrep_d[l][:])
                    btrep_t.append(btt)
                else:
                    btrep_t.append(None)
            idx_t = []
            for bb in range(NBANK):
                it0 = singles.tile([P, int(Cb[bb]) * 8], i16, name=f"idxr{bb}")
                nc.sync.dma_start(out=it0[:], in_=gidx_d[bb][:])
                idx_t.append(it0)
            iota_t = singles.tile([P, P], bf16)
            nc.sync.dma_start(out=iota_t[:], in_=iota_d[:])
            ident = singles.tile([P, P], f32)
            make_identity(nc, ident[:])
            eps_t = singles.tile([P, 1], f32)
            nc.vector.memset(eps_t[:], EPS)

            # bf16 weights for fast PE (cast once)
            wb_t = []
            for l in range(3):
                wb = singles.tile([P, D], bf16, name=f"wb{l}")
                nc.vector.tensor_copy(out=wb[:], in_=w_t[l][:])
                wb_t.append(wb)

            # per-layer AG input (own quarter) and gathered table, per quarter
            agin_d = [[dram.tile([QROWS, D], bf16, name=f"agin{l}_{q}")
                       for q in range(QT)] for l in range(3)]
            hfull_d = [[dram.tile([BANKROWS, D], bf16, addr_space="Shared",
                                  name=f"hfull{l}_{q}") for q in range(QT)]
                       for l in range(3)]

            HB = 5  # tiles per staging batch (25 % 5 == 0)

            def stage_tile(l, t, src_ap):
                """Scale rows of tile t by dinv_src, cast bf16, stage; DMA per
                HB-tile batch into agin_d[l][quarter]. src_ap: [P, D] f32/PSUM."""
                q, tq = divmod(t, QTILES)
                if tq % HB == 0:
                    stage_tile.buf = hstg.tile([P, HB, D], bf16, tag="hstage",
                                               name=f"hs{l}_{t}")
                nc.vector.scalar_tensor_tensor(
                    out=stage_tile.buf[:, tq % HB, :], in0=src_ap,
                    scalar=dinv_t[:, t:t + 1],
                    in1=w_t[0][:],
                    op0=mybir.AluOpType.mult, op1=mybir.AluOpType.bypass,
                )
                if tq % HB == HB - 1:
                    t0 = (tq // HB) * HB
                    nc.sync.dma_start(
                        out=agin_d[l][q][t0 * P:(t0 + HB) * P, :].rearrange(
                            "(c p) d -> p c d", p=P),
                        in_=stage_tile.buf[:],
                    )
                    if tq == QTILES - 1:
                        nc.gpsimd.collective_compute(
                            "AllGather",
                            mybir.AluOpType.bypass,
                            replica_groups=[list(range(NCORES))],
                            ins=[agin_d[l][q].opt()],
                            outs=[hfull_d[l][q].opt()],
                        )

            # ---- layer 0 phase A: h = x@W0 scaled, staged, quarter-AGs ----
            for t in range(TILES):
                hps = psmm.tile([P, D], f32, space="PSUM", tag="hps")
                nc.tensor.matmul(
                    out=hps[:],
                    lhsT=xcT[:, t * P:(t + 1) * P],
                    rhs=wb_t[0][:],
                    start=True, stop=True,
                )
                stage_tile(0, t, hps[:])

            # ---- layers: edge aggregation + per-tile tails ----
            for l in range(NLAYERS):
                gtiles = {}
                gq = 0
                stile = None
                acc = None
                for j in range(TOTCH):
                    t, bb, q = int(t_of[j]), int(b_of[j]), int(q_of[j])
                    grp, slot = divmod(q, GATHER_GROUP)
                    gk = (bb, grp)
                    if gk not in gtiles:
                        ng = min(GATHER_GROUP, int(Cb[bb]) - grp * GATHER_GROUP)
                        gt = gpool.tile([P, GATHER_GROUP, P], bf16, tag="gbuf",
                                        name=f"g{l}_{bb}_{grp}")
                        nc.gpsimd.dma_gather(
                            out_ap=gt[:, :ng, :],
                            in_ap=hfull_d[l][bb][:],
                            idxs_ap=idx_t[bb][:, grp * GATHER_GROUP * 8:
                                              (grp * GATHER_GROUP + ng) * 8],
                            num_idxs=ng * P,
                            num_idxs_reg=ng * P,
                            elem_size=P,
                            single_packet=False,
                            queue_num=gq % 4,
                        )
                        gq += 1
                        gtiles[gk] = gt
                    if j % S_BATCH == 0:
                        nb = min(S_BATCH, TOTCH - j)
                        stile = spool.tile([P, S_BATCH, P], bf16, tag="s",
                                           name=f"s{l}_{j}")
                        nc.vector.tensor_tensor(
                            out=stile[:, :nb, :],
                            in0=iota_t[:, None, :].to_broadcast([P, nb, P]),
                            in1=dstrel_t[:, j:j + nb].to_broadcast([P, nb, P]),
                            op=mybir.AluOpType.is_equal,
                        )
                    if is_first[j]:
                        acc = psacc.tile([P, D], f32, space="PSUM", tag="acc",
                                         name=f"acc{l}_{t}")
                    nc.tensor.matmul(
                        out=acc[:],
                        lhsT=stile[:, j % S_BATCH, :],
                        rhs=gtiles[gk][:, slot, :],
                        start=bool(is_first[j]),
                        stop=bool(is_last[j]),
                    )
                    if not is_last[j]:
                        continue

                    # ---- per-tile tail: LN (+ReLU), next-layer A or output ----
                    if triv["b"][l]:
                        conv = acc  # LN(dinv*acc + 0) == LN(acc)
                    else:
                        conv = lnp.tile([P, D], f32, tag="conv")
                        nc.vector.scalar_tensor_tensor(
                            out=conv[:], in0=acc[:],
                            scalar=dinv_t[:, t:t + 1],
                            in1=brep_t[l][:],
                            op0=mybir.AluOpType.mult,
                            op1=mybir.AluOpType.add,
                        )
                    stats = lnp.tile([P, 6], f32, tag="stats")
                    nc.vector.bn_stats(out=stats[:], in_=conv[:])
                    mv = lnp.tile([P, 2], f32, tag="mv")
                    nc.vector.bn_aggr(out=mv[:], in_=stats[:])
                    rstd = lnp.tile([P, 1], f32, tag="rstd")
                    nc.scalar.activation(
                        out=rstd[:], in_=mv[:, 1:2],
                        func=mybir.ActivationFunctionType.Sqrt,
                        bias=eps_t[:],
                    )
                    nc.vector.reciprocal(out=rstd[:], in_=rstd[:])
                    y = lnp.tile([P, D], f32, tag="y")
                    nc.vector.scalar_tensor_tensor(
                        out=y[:], in0=conv[:], scalar=mv[:, 0:1],
                        in1=rstd[:].to_broadcast([P, D]),
                        op0=mybir.AluOpType.subtract,
                        op1=mybir.AluOpType.mult,
                    )
                    if not triv["g"][l]:
                        nc.vector.tensor_mul(out=y[:], in0=y[:], in1=grep_t[l][:])
                    if not triv["bt"][l]:
                        nc.vector.tensor_add(out=y[:], in0=y[:], in1=btrep_t[l][:])

                    if l == NLAYERS - 1:
                        nc.sync.dma_start(
                            out=out_d[t * P:(t + 1) * P, :], in_=y[:])
                        continue
                    nc.scalar.activation(
                        out=y[:], in_=y[:],
                        func=mybir.ActivationFunctionType.Relu,
                    )
                    tp = pstp.tile([P, P], f32, space="PSUM", tag="tp")
                    nc.tensor.transpose(out=tp[:], in_=y[:], identity=ident[:])
                    nc.scalar.copy(out=xcT[:, t * P:(t + 1) * P], in_=tp[:])
                    # next layer phase A for this tile
                    hps = psmm.tile([P, D], f32, space="PSUM", tag="hps")
                    nc.tensor.matmul(
                        out=hps[:],
                        lhsT=xcT[:, t * P:(t + 1) * P],
                        rhs=wb_t[l + 1][:],
                        start=True, stop=True,
                    )
                    stage_tile(l + 1, t, hps[:])

                if l < NLAYERS - 1:
                    # pad tiles (no chunks): their xcT stays zero; stage zeros
                    for t in range(TILES):
                        if t in tiles_with_chunks:
                            continue
                        hps = psmm.tile([P, D], f32, space="PSUM", tag="hps")
                        nc.tensor.matmul(
                            out=hps[:],
                            lhsT=xcT[:, t * P:(t + 1) * P],
                            rhs=wb_t[l + 1][:],
                            start=True, stop=True,
                        )
                        stage_tile(l + 1, t, hps[:])

    nc.compile()
    return nc


def _ensure_ntff_hook():
    import types

    try:
        from antenv.axon_hooks import get_axon_ntff_profile_hook  # noqa: F401
        return
    except ImportError:
        pass
    import antenv

    mod = types.ModuleType("antenv.axon_hooks")
    mod._hook = None

    def set_axon_ntff_profile_hook(h):
        mod._hook = h

    def get_axon_ntff_profile_hook():
        return mod._hook

    mod.set_axon_ntff_profile_hook = set_axon_ntff_profile_hook
    mod.get_axon_ntff_profile_hook = get_axon_ntff_profile_hook
    sys.modules["antenv.axon_hooks"] = mod
    antenv.axon_hooks = mod
    try:
        from trn_agent_boot.trn_boot import _ntff_profile_via_ctypes

        mod._hook = _ntff_profile_via_ctypes("/opt/axon/libaxon_pjrt.so")
    except Exception as e:
        print("ntff hook setup failed:", e)


def kernel(**inputs) -> np.ndarray:
    x = np.asarray(inputs["x"], np.float32)
    edge_index = np.asarray(inputs["edge_index"])
    Ws = [np.asarray(inputs[f"W{l}"], np.float32) for l in range(3)]
    bs = [np.asarray(inputs[f"b{l}"], np.float32) for l in range(3)]
    gs = [np.asarray(inputs[f"g{l}"], np.float32) for l in range(3)]
    bts = [np.asarray(inputs[f"bt{l}"], np.float32) for l in range(3)]

    triv = dict(
        b=[bool(np.all(b == 0)) for b in bs],
        g=[bool(np.all(g == 1)) for g in gs],
        bt=[bool(np.all(bt == 0)) for bt in bts],
    )

    sched, xcT, dinv_in, dstrel_in, gidx = _preprocess(x, edge_index)
    nc = _build(sched, sched["Cb"], triv)

    iota = np.broadcast_to(
        np.arange(P, dtype=np.float32), (P, P)
    ).astype(ml_dtypes.bfloat16)

    in_maps = []
    for c in range(NCORES):
        m = dict(
            xcT=np.ascontiguousarray(xcT[c]),
            dinv=np.ascontiguousarray(dinv_in[c]),
            dstrel=np.ascontiguousarray(dstrel_in[c]),
            iota=np.ascontiguousarray(iota),
        )
        for bb in range(NBANK):
            m[f"gidx{bb}"] = np.ascontiguousarray(gidx[bb][c])
        for l in range(3):
            m[f"w{l}"] = Ws[l]
            m[f"brep{l}"] = np.ascontiguousarray(
                np.broadcast_to(bs[l], (P, D)).astype(np.float32))
            m[f"grep{l}"] = np.ascontiguousarray(
                np.broadcast_to(gs[l], (P, D)).astype(np.float32))
            m[f"btrep{l}"] = np.ascontiguousarray(
                np.broadcast_to(bts[l], (P, D)).astype(np.float32))
        in_maps.append(m)

    from concourse.bass_utils import run_bass_kernel_spmd

    trace = bool(int(os.environ.get("GCN_TRACE", "0")))
    if trace:
        _ensure_ntff_hook()
    res = run_bass_kernel_spmd(
        nc, in_maps, core_ids=list(range(NCORES)), trace=trace
    )
    kernel.last_results = res

    out = np.zeros((N, D), np.float32)
    core_of = sched["core_of"]
    sidx_of = sched["sidx_of"]
    for c in range(NCORES):
        mask = core_of == c
        out[mask] = res.results[c]["out"][sidx_of[mask]]
    return out


# revision 4
# speedup vs baseline: 1.0469x; 1.0278x over previous
"""3-layer GCN encoder on 8 TRN2 NeuronCores — v2.

Strategy (dst-partitioned graph parallel, pipelined):
  - Nodes partitioned across 8 cores (12500 each, padded to NPAD=12800,
    TILES=100 tiles of 128).
  - Per layer, each core holds the full scaled source table
    hfull [8*NPAD, 128] bf16, built by 4 quarter-AllGathers (each quarter =
    25 tiles = 3200 rows per core -> bank of 8*3200=25600 rows,
    int16-addressable).
  - Edge phase: edges grouped by (dst_tile, src_bank); 128-edge chunks
    gathered via dma_gather (4 SWDGE queues) and scatter-added into the dst
    tile's PSUM via one-hot matmuls (S built with is_equal in batches).
  - Per-tile tail: LayerNorm directly on the PSUM accumulator (the dst-side
    deg^-1/2 scale and bias b fold away via LN affine invariance when b==0;
    general path emitted if b!=0), then ReLU, then transpose (PE) and the
    NEXT layer's x@W matmul + dinv_src scale + bf16 staging, so phase A of
    layer l+1 is hidden inside layer l's edge phase. Quarter-AllGathers for
    layer l+1 fire as soon as their 25 tiles are staged.
  - Layer 2's tail writes the output slice instead.

kernel(**inputs) takes FULL inputs, returns the FULL [100000, 128] f32 output.
"""
import os
import sys

sys.path.insert(0, "/opt/trn_rl_repo")

import numpy as np
import ml_dtypes

N = 100000
D = 128
NCORES = 8
P = 128
TILES = 100
NPAD = TILES * P          # 12800
QT = 4                    # quarters (AllGather splits)
QTILES = TILES // QT      # 25 tiles per quarter
QROWS = QTILES * P        # 3200 rows per core per quarter
BANKROWS = NCORES * QROWS # 25600 rows per bank (< 32767)
NBANK = QT
EPS = 1e-5

GATHER_GROUP = int(os.environ.get("GCN_G", "64"))   # chunks per dma_gather
S_BATCH = int(os.environ.get("GCN_SB", "16"))       # chunks per is_equal
GBUFS = int(os.environ.get("GCN_GBUFS", "4"))
NLAYERS = 3


def _preprocess(x, edge_index):
    """Host-side graph preprocessing -> per-core arrays + shared schedule."""
    ei = np.asarray(edge_index)
    # self-loops are NOT materialized as edges: each tile's self-messages are
    # seeded into PSUM by a matmul over the dinv-scaled xcT tile instead.
    src = ei[0].astype(np.int64)
    dst = ei[1].astype(np.int64)
    M = src.shape[0]

    deg = (np.bincount(dst, minlength=N) + 1).astype(np.float32)  # + self-loop
    dinv = 1.0 / np.sqrt(deg)

    # degree-sorted global tiles, round-robin over cores
    p_of = np.empty(N, np.int64)
    p_of[np.argsort(-deg, kind="stable")] = np.arange(N)
    gtile = p_of >> 7
    slot_of = gtile // NCORES

    # Balance cores within each slot-octet: a node's source-quarter depends
    # only on its slot, so per-slot core reassignment leaves every node's
    # bank invariant while equalizing cnt[core, tile, bank] across cores
    # (shrinking the max-over-cores chunk padding).
    q_src = np.minimum(slot_of // QTILES, QT - 1)
    nvec = np.zeros((N, NBANK), np.int32)     # in-degree by source quarter
    np.add.at(nvec, (dst, q_src[src]), 1)
    tot_in = nvec.sum(axis=1)

    core_of = np.empty(N, np.int64)
    pos_of = np.empty(N, np.int64)
    order_slot = np.argsort(slot_of, kind="stable")
    bounds = np.searchsorted(slot_of[order_slot], np.arange(slot_of.max() + 2))
    for s in range(slot_of.max() + 1):
        nodes = order_slot[bounds[s]:bounds[s + 1]]
        nodes = nodes[np.argsort(-tot_in[nodes], kind="stable")]
        S = np.zeros((NCORES, NBANK), np.int64)  # bank-count sums
        caps = np.full(NCORES, P, np.int64)
        filled = np.zeros(NCORES, np.int64)
        for i in nodes:
            v = nvec[i]
            Sv = S + v
            cost = (Sv * Sv).sum(axis=1).astype(np.float64)
            cost[caps == 0] = np.inf
            c = int(np.argmin(cost))
            core_of[i] = c
            pos_of[i] = filled[c]
            filled[c] += 1
            caps[c] -= 1
            S[c] += v
    sidx_of = slot_of * P + pos_of          # row within the core's slice

    # bank = quarter of the source's slice; brel = core*QROWS + row-in-quarter
    q_of_node = sidx_of // QROWS
    brel_of_node = core_of * QROWS + (sidx_of % QROWS)

    core = core_of[dst]
    t = slot_of[dst]
    drel = pos_of[dst]
    b = q_of_node[src]
    srel = brel_of_node[src]

    key = (core * TILES + t) * NBANK + b
    # sort by group, then by source row within group (HBM locality)
    order = np.lexsort((srel, key))
    key_s = key[order]
    core_s = core[order]
    srel_s = srel[order]
    drel_s = drel[order]

    cnt = np.bincount(key, minlength=NCORES * TILES * NBANK).reshape(
        NCORES, TILES, NBANK
    )
    K = np.ceil(cnt.max(axis=0) / P).astype(np.int64)  # [TILES, NBANK]
    Ltb = (K * P).reshape(-1)
    off2 = np.concatenate([[0], np.cumsum(Ltb)[:-1]])
    TOT = int(Ltb.sum())
    TOTCH = TOT // P

    first = np.searchsorted(key_s, key_s, side="left")
    rank = np.arange(M) - first
    pos = off2[(key_s % (TILES * NBANK))] + rank

    srcrel_pad = np.full((NCORES, TOT), -1, np.int64)
    dstrel_pad = np.full((NCORES, TOT), -1.0, np.float32)
    srcrel_pad[core_s, pos] = srel_s
    dstrel_pad[core_s, pos] = drel_s.astype(np.float32)
    # pad slots: duplicate the previous real index (page locality, no garbage)
    for c in range(NCORES):
        row = srcrel_pad[c]
        bad = row < 0
        if bad.any():
            idxs = np.where(~bad, np.arange(TOT), 0)
            np.maximum.accumulate(idxs, out=idxs)
            row[:] = row[idxs]
            row[row < 0] = 0
    srcrel_pad = srcrel_pad.astype(np.int16)

    # chunk schedule: chunk j -> (t, b), bank stream position q
    tb_of_chunk = np.repeat(np.arange(TILES * NBANK), K.reshape(-1))
    t_of_chunk = tb_of_chunk // NBANK
    b_of_chunk = tb_of_chunk % NBANK
    q_of_chunk = np.zeros(TOTCH, np.int64)
    Cb = np.zeros(NBANK, np.int64)
    for j in range(TOTCH):
        bb = b_of_chunk[j]
        q_of_chunk[j] = Cb[bb]
        Cb[bb] += 1

    # per-bank idx streams, wrapped int16 [128, C_b * 8]
    gidx = []
    chunks_src = srcrel_pad.reshape(NCORES, TOTCH, P)
    for bb in range(NBANK):
        sel = chunks_src[:, b_of_chunk == bb, :].reshape(NCORES, -1)
        w = sel.reshape(NCORES, -1, 16).transpose(0, 2, 1)
        gidx.append(np.tile(w, (1, 8, 1)).astype(np.int16))

    dstrel_in = dstrel_pad.reshape(NCORES, TOTCH, P).transpose(0, 2, 1)
    dstrel_in = dstrel_in.astype(ml_dtypes.bfloat16)

    # per-core x feature-major bf16, and per-source dinv
    x = np.asarray(x, dtype=np.float32)
    x_pad = np.zeros((NCORES, NPAD, D), np.float32)
    x_pad[core_of, sidx_of] = x * dinv[:, None]   # fold src-side deg^-1/2
    xcT = np.ascontiguousarray(x_pad.transpose(0, 2, 1)).astype(
        ml_dtypes.bfloat16
    )  # [8, 128, 12800]

    dinv_pad = np.zeros((NCORES, NPAD), np.float32)
    dinv_pad[core_of, sidx_of] = dinv
    dinv_in = np.ascontiguousarray(
        dinv_pad.reshape(NCORES, TILES, P).transpose(0, 2, 1)
    )  # [8, 128, TILES]

    sched = dict(
        K=K, TOTCH=TOTCH, t_of_chunk=t_of_chunk, b_of_chunk=b_of_chunk,
        q_of_chunk=q_of_chunk, Cb=Cb, core_of=core_of, sidx_of=sidx_of,
    )
    return sched, xcT, dinv_in, dstrel_in, gidx


def _build(sched, Cb, triv):
    """triv: dict(b=[bool]*3, g=[bool]*3, bt=[bool]*3) — which params are
    trivial (b==0, g==1, bt==0), letting ops be elided at trace time."""
    from concourse import bass, bacc, mybir, tile
    from concourse.masks import make_identity

    f32 = mybir.dt.float32
    bf16 = mybir.dt.bfloat16
    i16 = mybir.dt.int16

    TOTCH = sched["TOTCH"]
    t_of = sched["t_of_chunk"]
    b_of = sched["b_of_chunk"]
    q_of = sched["q_of_chunk"]

    is_first = np.zeros(TOTCH, bool)
    is_last = np.zeros(TOTCH, bool)
    prev_t = -1
    for j in range(TOTCH):
        if t_of[j] != prev_t:
            is_first[j] = True
            if j > 0:
                is_last[j - 1] = True
            prev_t = t_of[j]
    is_last[TOTCH - 1] = True
    # tiles with no chunks at all (pad tiles)
    tiles_with_chunks = sorted(set(int(t) for t in t_of))

    nc = bacc.Bacc("TRN2", debug=False, num_devices=NCORES, num_swdge_queues=4)

    xcT_d = nc.dram_tensor("xcT", [P, NPAD], bf16, kind="ExternalInput")
    dinv_d = nc.dram_tensor("dinv", [P, TILES], f32, kind="ExternalInput")
    dstrel_d = nc.dram_tensor("dstrel", [P, TOTCH], bf16, kind="ExternalInput")
    gidx_d = [
        nc.dram_tensor(f"gidx{bb}", [P, int(Cb[bb]) * 8], i16, kind="ExternalInput")
        for bb in range(NBANK)
    ]
    w_d = [nc.dram_tensor(f"w{l}", [P, D], f32, kind="ExternalInput") for l in range(3)]
    brep_d = [nc.dram_tensor(f"brep{l}", [P, D], f32, kind="ExternalInput")
              for l in range(3)]
    grep_d = [nc.dram_tensor(f"grep{l}", [P, D], f32, kind="ExternalInput")
              for l in range(3)]
    btrep_d = [nc.dram_tensor(f"btrep{l}", [P, D], f32, kind="ExternalInput")
               for l in range(3)]
    iota_d = nc.dram_tensor("iota", [P, P], bf16, kind="ExternalInput")
    out_d = nc.dram_tensor("out", [NPAD, D], f32, kind="ExternalOutput")

    with tile.TileContext(nc) as tc:
        with (
            tc.tile_pool(name="singles", bufs=1) as singles,
            tc.tile_pool(name="gpool", bufs=GBUFS) as gpool,
            tc.tile_pool(name="spool", bufs=3) as spool,
            tc.tile_pool(name="hstg", bufs=3) as hstg,
            tc.tile_pool(name="ln", bufs=3) as lnp,
            tc.tile_pool(name="psacc", bufs=4, space="PSUM") as psacc,
            tc.tile_pool(name="psmm", bufs=2, space="PSUM") as psmm,
            tc.tile_pool(name="pstp", bufs=2, space="PSUM") as pstp,
            tc.tile_pool(name="dram", bufs=1, space="DRAM") as dram,
        ):
            # ---- persistent SBUF state ----
            xcT = singles.tile([P, NPAD], bf16)
            nc.sync.dma_start(out=xcT[:], in_=xcT_d[:])
            dinv_t = singles.tile([P, TILES], f32)
            nc.sync.dma_start(out=dinv_t[:], in_=dinv_d[:])
            dstrel_t = singles.tile([P, TOTCH], bf16)
            nc.sync.dma_start(out=dstrel_t[:], in_=dstrel_d[:])

            w_t, brep_t, grep_t, btrep_t = [], [], [], []
            for l in range(3):
                wt = singles.tile([P, D], f32, name=f"w{l}")
                nc.sync.dma_start(out=wt[:], in_=w_d[l][:])
                w_t.append(wt)
                if not triv["b"][l]:
                    bt = singles.tile([P, D], f32, name=f"brep{l}")
                    nc.sync.dma_start(out=bt[:], in_=brep_d[l][:])
                    brep_t.append(bt)
                else:
                    brep_t.append(None)
                if not triv["g"][l]:
                    gt = singles.tile([P, D], f32, name=f"grep{l}")
                    nc.sync.dma_start(out=gt[:], in_=grep_d[l][:])
                    grep_t.append(gt)
                else:
                    grep_t.append(None)
                if not triv["bt"][l]:
                    btt = singles.tile([P, D], f32, name=f"btrep{l}")
                    nc.sync.dma_start(out=btt[:], in_=btrep_d[l][:])
                    btrep_t.append(btt)
                else:
                    btrep_t.append(None)
            idx_t = []
            for bb in range(NBANK):
                it0 = singles.tile([P, int(Cb[bb]) * 8], i16, name=f"idxr{bb}")
                nc.sync.dma_start(out=it0[:], in_=gidx_d[bb][:])
                idx_t.append(it0)
            iota_t = singles.tile([P, P], bf16)
            nc.sync.dma_start(out=iota_t[:], in_=iota_d[:])
            ident = singles.tile([P, P], f32)
            make_identity(nc, ident[:])
            eps_t = singles.tile([P, 1], f32)
            nc.vector.memset(eps_t[:], EPS)

            # bf16 weights for fast PE (cast once)
            wb_t = []
            for l in range(3):
                wb = singles.tile([P, D], bf16, name=f"wb{l}")
                nc.vector.tensor_copy(out=wb[:], in_=w_t[l][:])
                wb_t.append(wb)

            # per-layer AG input (own quarter) and gathered table, per quarter
            agin_d = [[dram.tile([QROWS, D], bf16, name=f"agin{l}_{q}")
                       for q in range(QT)] for l in range(3)]
            hfull_d = [[dram.tile([BANKROWS, D], bf16, addr_space="Shared",
                                  name=f"hfull{l}_{q}") for q in range(QT)]
                       for l in range(3)]

            HB = 5  # tiles per staging batch (25 % 5 == 0)

            def stage_tile(l, t, src_ap):
                """Scale rows of tile t by dinv_src, cast bf16, stage; DMA per
                HB-tile batch into agin_d[l][quarter]. src_ap: [P, D] f32/PSUM."""
                q, tq = divmod(t, QTILES)
                if tq % HB == 0:
                    stage_tile.buf = hstg.tile([P, HB, D], bf16, tag="hstage",
                                               name=f"hs{l}_{t}")
                nc.vector.tensor_copy(
                    out=stage_tile.buf[:, tq % HB, :], in_=src_ap)
                if tq % HB == HB - 1:
                    t0 = (tq // HB) * HB
                    nc.sync.dma_start(
                        out=agin_d[l][q][t0 * P:(t0 + HB) * P, :].rearrange(
                            "(c p) d -> p c d", p=P),
                        in_=stage_tile.buf[:],
                    )
                    if tq == QTILES - 1:
                        nc.gpsimd.collective_compute(
                            "AllGather",
                            mybir.AluOpType.bypass,
                            replica_groups=[list(range(NCORES))],
                            ins=[agin_d[l][q].opt()],
                            outs=[hfull_d[l][q].opt()],
                        )

            # ---- layer 0 phase A: h = x@W0 scaled, staged, quarter-AGs ----
            for t in range(TILES):
                hps = psmm.tile([P, D], f32, space="PSUM", tag="hps")
                nc.tensor.matmul(
                    out=hps[:],
                    lhsT=xcT[:, t * P:(t + 1) * P],
                    rhs=wb_t[0][:],
                    start=True, stop=True,
                )
                stage_tile(0, t, hps[:])

            # ---- layers: edge aggregation + per-tile tails ----
            for l in range(NLAYERS):
                gtiles = {}
                gq = 0
                stile = None
                acc = None
                for j in range(TOTCH):
                    t, bb, q = int(t_of[j]), int(b_of[j]), int(q_of[j])
                    grp, slot = divmod(q, GATHER_GROUP)
                    gk = (bb, grp)
                    if gk not in gtiles:
                        ng = min(GATHER_GROUP, int(Cb[bb]) - grp * GATHER_GROUP)
                        gt = gpool.tile([P, GATHER_GROUP, P], bf16, tag="gbuf",
                                        name=f"g{l}_{bb}_{grp}")
                        nc.gpsimd.dma_gather(
                            out_ap=gt[:, :ng, :],
                            in_ap=hfull_d[l][bb][:],
                            idxs_ap=idx_t[bb][:, grp * GATHER_GROUP * 8:
                                              (grp * GATHER_GROUP + ng) * 8],
                            num_idxs=ng * P,
                            num_idxs_reg=ng * P,
                            elem_size=P,
                            single_packet=False,
                            queue_num=gq % 4,
                        )
                        gq += 1
                        gtiles[gk] = gt
                    if j % S_BATCH == 0:
                        nb = min(S_BATCH, TOTCH - j)
                        stile = spool.tile([P, S_BATCH, P], bf16, tag="s",
                                           name=f"s{l}_{j}")
                        nc.vector.tensor_tensor(
                            out=stile[:, :nb, :],
                            in0=iota_t[:, None, :].to_broadcast([P, nb, P]),
                            in1=dstrel_t[:, j:j + nb].to_broadcast([P, nb, P]),
                            op=mybir.AluOpType.is_equal,
                        )
                    if is_first[j]:
                        acc = psacc.tile([P, D], f32, space="PSUM", tag="acc",
                                         name=f"acc{l}_{t}")
                        # seed with this tile's self-messages (scaled x@W)
                        nc.tensor.matmul(
                            out=acc[:],
                            lhsT=xcT[:, t * P:(t + 1) * P],
                            rhs=wb_t[l][:],
                            start=True, stop=False,
                        )
                    nc.tensor.matmul(
                        out=acc[:],
                        lhsT=stile[:, j % S_BATCH, :],
                        rhs=gtiles[gk][:, slot, :],
                        start=False,
                        stop=bool(is_last[j]),
                    )
                    if not is_last[j]:
                        continue

                    # ---- per-tile tail: LN (+ReLU), next-layer A or output ----
                    if triv["b"][l]:
                        conv = acc  # LN(dinv*acc + 0) == LN(acc)
                    else:
                        conv = lnp.tile([P, D], f32, tag="conv")
                        nc.vector.scalar_tensor_tensor(
                            out=conv[:], in0=acc[:],
                            scalar=dinv_t[:, t:t + 1],
                            in1=brep_t[l][:],
                            op0=mybir.AluOpType.mult,
                            op1=mybir.AluOpType.add,
                        )
                    stats = lnp.tile([P, 6], f32, tag="stats")
                    nc.vector.bn_stats(out=stats[:], in_=conv[:])
                    mv = lnp.tile([P, 2], f32, tag="mv")
                    nc.vector.bn_aggr(out=mv[:], in_=stats[:])
                    rstd = lnp.tile([P, 1], f32, tag="rstd")
                    nc.scalar.activation(
                        out=rstd[:], in_=mv[:, 1:2],
                        func=mybir.ActivationFunctionType.Sqrt,
                        bias=eps_t[:],
                    )
                    nc.vector.reciprocal(out=rstd[:], in_=rstd[:])
                    y = lnp.tile([P, D], f32, tag="y")
                    nc.vector.scalar_tensor_tensor(
                        out=y[:], in0=conv[:], scalar=mv[:, 0:1],
                        in1=rstd[:].to_broadcast([P, D]),
                        op0=mybir.AluOpType.subtract,
                        op1=mybir.AluOpType.mult,
                    )
                    if not triv["g"][l]:
                        nc.vector.tensor_mul(out=y[:], in0=y[:], in1=grep_t[l][:])
                    if not triv["bt"][l]:
                        nc.vector.tensor_add(out=y[:], in0=y[:], in1=btrep_t[l][:])

                    if l == NLAYERS - 1:
                        nc.sync.dma_start(
                            out=out_d[t * P:(t + 1) * P, :], in_=y[:])
                        continue
                    nc.scalar.activation(
                        out=y[:], in_=y[:],
                        func=mybir.ActivationFunctionType.Relu,
                        scale=dinv_t[:, t:t + 1],
                    )
                    tp = pstp.tile([P, P], f32, space="PSUM", tag="tp")
                    nc.tensor.transpose(out=tp[:], in_=y[:], identity=ident[:])
                    nc.scalar.copy(out=xcT[:, t * P:(t + 1) * P], in_=tp[:])
                    # next layer phase A for this tile
                    hps = psmm.tile([P, D], f32, space="PSUM", tag="hps")
                    nc.tensor.matmul(
                        out=hps[:],
                        lhsT=xcT[:, t * P:(t + 1) * P],
                        rhs=wb_t[l + 1][:],
                        start=True, stop=True,
                    )
                    stage_tile(l + 1, t, hps[:])

                if l < NLAYERS - 1:
                    # pad tiles (no chunks): their xcT stays zero; stage zeros
                    for t in range(TILES):
                        if t in tiles_with_chunks:
                            continue
                        hps = psmm.tile([P, D], f32, space="PSUM", tag="hps")
                        nc.tensor.matmul(
                            out=hps[:],
                            lhsT=xcT[:, t * P:(t + 1) * P],
                            rhs=wb_t[l + 1][:],
                            start=True, stop=True,
                        )
                        stage_tile(l + 1, t, hps[:])

    nc.compile()
    return nc


def _ensure_ntff_hook():
    import types

    try:
        from antenv.axon_hooks import get_axon_ntff_profile_hook  # noqa: F401
        return
    except ImportError:
        pass
    import antenv

    mod = types.ModuleType("antenv.axon_hooks")
    mod._hook = None

    def set_axon_ntff_profile_hook(h):
        mod._hook = h

    def get_axon_ntff_profile_hook():
        return mod._hook

    mod.set_axon_ntff_profile_hook = set_axon_ntff_profile_hook
    mod.get_axon_ntff_profile_hook = get_axon_ntff_profile_hook
    sys.modules["antenv.axon_hooks"] = mod
    antenv.axon_hooks = mod
    try:
        from trn_agent_boot.trn_boot import _ntff_profile_via_ctypes

        mod._hook = _ntff_profile_via_ctypes("/opt/axon/libaxon_pjrt.so")
    except Exception as e:
        print("ntff hook setup failed:", e)


def kernel(**inputs) -> np.ndarray:
    x = np.asarray(inputs["x"], np.float32)
    edge_index = np.asarray(inputs["edge_index"])
    Ws = [np.asarray(inputs[f"W{l}"], np.float32) for l in range(3)]
    bs = [np.asarray(inputs[f"b{l}"], np.float32) for l in range(3)]
    gs = [np.asarray(inputs[f"g{l}"], np.float32) for l in range(3)]
    bts = [np.asarray(inputs[f"bt{l}"], np.float32) for l in range(3)]

    triv = dict(
        b=[bool(np.all(b == 0)) for b in bs],
        g=[bool(np.all(g == 1)) for g in gs],
        bt=[bool(np.all(bt == 0)) for bt in bts],
    )

    sched, xcT, dinv_in, dstrel_in, gidx = _preprocess(x, edge_index)
    nc = _build(sched, sched["Cb"], triv)

    iota = np.broadcast_to(
        np.arange(P, dtype=np.float32), (P, P)
    ).astype(ml_dtypes.bfloat16)

    in_maps = []
    for c in range(NCORES):
        m = dict(
            xcT=np.ascontiguousarray(xcT[c]),
            dinv=np.ascontiguousarray(dinv_in[c]),
            dstrel=np.ascontiguousarray(dstrel_in[c]),
            iota=np.ascontiguousarray(iota),
        )
        for bb in range(NBANK):
            m[f"gidx{bb}"] = np.ascontiguousarray(gidx[bb][c])
        for l in range(3):
            m[f"w{l}"] = Ws[l]
            m[f"brep{l}"] = np.ascontiguousarray(
                np.broadcast_to(bs[l], (P, D)).astype(np.float32))
            m[f"grep{l}"] = np.ascontiguousarray(
                np.broadcast_to(gs[l], (P, D)).astype(np.float32))
            m[f"btrep{l}"] = np.ascontiguousarray(
                np.broadcast_to(bts[l], (P, D)).astype(np.float32))
        in_maps.append(m)

    from concourse.bass_utils import run_bass_kernel_spmd

    trace = bool(int(os.environ.get("GCN_TRACE", "0")))
    if trace:
        _ensure_ntff_hook()
    res = run_bass_kernel_spmd(
        nc, in_maps, core_ids=list(range(NCORES)), trace=trace
    )
    kernel.last_results = res

    out = np.zeros((N, D), np.float32)
    core_of = sched["core_of"]
    sidx_of = sched["sidx_of"]
    for c in range(NCORES):
        mask = core_of == c
        out[mask] = res.results[c]["out"][sidx_of[mask]]
    return out


# revision 6
# speedup vs baseline: 1.0539x; 1.0067x over previous
"""3-layer GCN encoder on 8 TRN2 NeuronCores — v2.

Strategy (dst-partitioned graph parallel, pipelined):
  - Nodes partitioned across 8 cores (12500 each, padded to NPAD=12800,
    TILES=100 tiles of 128).
  - Per layer, each core holds the full scaled source table
    hfull [8*NPAD, 128] bf16, built by 4 quarter-AllGathers (each quarter =
    25 tiles = 3200 rows per core -> bank of 8*3200=25600 rows,
    int16-addressable).
  - Edge phase: edges grouped by (dst_tile, src_bank); 128-edge chunks
    gathered via dma_gather (4 SWDGE queues) and scatter-added into the dst
    tile's PSUM via one-hot matmuls (S built with is_equal in batches).
  - Per-tile tail: LayerNorm directly on the PSUM accumulator (the dst-side
    deg^-1/2 scale and bias b fold away via LN affine invariance when b==0;
    general path emitted if b!=0), then ReLU, then transpose (PE) and the
    NEXT layer's x@W matmul + dinv_src scale + bf16 staging, so phase A of
    layer l+1 is hidden inside layer l's edge phase. Quarter-AllGathers for
    layer l+1 fire as soon as their 25 tiles are staged.
  - Layer 2's tail writes the output slice instead.

kernel(**inputs) takes FULL inputs, returns the FULL [100000, 128] f32 output.
"""
import os
import sys

sys.path.insert(0, "/opt/trn_rl_repo")

import numpy as np
import ml_dtypes

N = 100000
D = 128
NCORES = 8
P = 128
TILES = 100
NPAD = TILES * P          # 12800
QT = 4                    # quarters (AllGather splits)
QTILES = TILES // QT      # 25 tiles per quarter
QROWS = QTILES * P        # 3200 rows per core per quarter
BANKROWS = NCORES * QROWS # 25600 rows per bank (< 32767)
NBANK = QT
EPS = 1e-5

GATHER_GROUP = int(os.environ.get("GCN_G", "32"))   # chunks per dma_gather
S_BATCH = int(os.environ.get("GCN_SB", "16"))       # chunks per is_equal
GBUFS = int(os.environ.get("GCN_GBUFS", "8"))
NLAYERS = 3


def _preprocess(x, edge_index):
    """Host-side graph preprocessing -> per-core arrays + shared schedule."""
    ei = np.asarray(edge_index)
    # self-loops are NOT materialized as edges: each tile's self-messages are
    # seeded into PSUM by a matmul over the dinv-scaled xcT tile instead.
    src = ei[0].astype(np.int64)
    dst = ei[1].astype(np.int64)
    M = src.shape[0]

    deg = (np.bincount(dst, minlength=N) + 1).astype(np.float32)  # + self-loop
    dinv = 1.0 / np.sqrt(deg)

    # degree-sorted global tiles, round-robin over cores
    p_of = np.empty(N, np.int64)
    p_of[np.argsort(-deg, kind="stable")] = np.arange(N)
    gtile = p_of >> 7
    slot_of = gtile // NCORES

    # Balance cores within each slot-octet: a node's source-quarter depends
    # only on its slot, so per-slot core reassignment leaves every node's
    # bank invariant while equalizing cnt[core, tile, bank] across cores
    # (shrinking the max-over-cores chunk padding).
    q_src = np.minimum(slot_of // QTILES, QT - 1)
    nvec = np.zeros((N, NBANK), np.int32)     # in-degree by source quarter
    np.add.at(nvec, (dst, q_src[src]), 1)
    tot_in = nvec.sum(axis=1)

    core_of = np.empty(N, np.int64)
    pos_of = np.empty(N, np.int64)
    order_slot = np.argsort(slot_of, kind="stable")
    bounds = np.searchsorted(slot_of[order_slot], np.arange(slot_of.max() + 2))
    for s in range(slot_of.max() + 1):
        nodes = order_slot[bounds[s]:bounds[s + 1]]
        nodes = nodes[np.argsort(-tot_in[nodes], kind="stable")]
        S = np.zeros((NCORES, NBANK), np.int64)  # bank-count sums
        caps = np.full(NCORES, P, np.int64)
        filled = np.zeros(NCORES, np.int64)
        for i in nodes:
            v = nvec[i]
            Sv = S + v
            cost = (Sv * Sv).sum(axis=1).astype(np.float64)
            cost[caps == 0] = np.inf
            c = int(np.argmin(cost))
            core_of[i] = c
            pos_of[i] = filled[c]
            filled[c] += 1
            caps[c] -= 1
            S[c] += v
    sidx_of = slot_of * P + pos_of          # row within the core's slice

    # bank = quarter of the source's slice; brel = core*QROWS + row-in-quarter
    q_of_node = sidx_of // QROWS
    brel_of_node = core_of * QROWS + (sidx_of % QROWS)

    core = core_of[dst]
    t = slot_of[dst]
    drel = pos_of[dst]
    b = q_of_node[src]
    srel = brel_of_node[src]

    key = (core * TILES + t) * NBANK + b
    # sort by group, then by source row within group (HBM locality)
    order = np.lexsort((srel, key))
    key_s = key[order]
    core_s = core[order]
    srel_s = srel[order]
    drel_s = drel[order]

    cnt = np.bincount(key, minlength=NCORES * TILES * NBANK).reshape(
        NCORES, TILES, NBANK
    )
    K = np.ceil(cnt.max(axis=0) / P).astype(np.int64)  # [TILES, NBANK]
    Ltb = (K * P).reshape(-1)
    off2 = np.concatenate([[0], np.cumsum(Ltb)[:-1]])
    TOT = int(Ltb.sum())
    TOTCH = TOT // P

    first = np.searchsorted(key_s, key_s, side="left")
    rank = np.arange(M) - first
    pos = off2[(key_s % (TILES * NBANK))] + rank

    srcrel_pad = np.full((NCORES, TOT), -1, np.int64)
    dstrel_pad = np.full((NCORES, TOT), -1.0, np.float32)
    srcrel_pad[core_s, pos] = srel_s
    dstrel_pad[core_s, pos] = drel_s.astype(np.float32)
    # pad slots: duplicate the previous real index (page locality, no garbage)
    for c in range(NCORES):
        row = srcrel_pad[c]
        bad = row < 0
        if bad.any():
            idxs = np.where(~bad, np.arange(TOT), 0)
            np.maximum.accumulate(idxs, out=idxs)
            row[:] = row[idxs]
            row[row < 0] = 0
    srcrel_pad = srcrel_pad.astype(np.int16)

    # chunk schedule: chunk j -> (t, b), bank stream position q
    tb_of_chunk = np.repeat(np.arange(TILES * NBANK), K.reshape(-1))
    t_of_chunk = tb_of_chunk // NBANK
    b_of_chunk = tb_of_chunk % NBANK
    q_of_chunk = np.zeros(TOTCH, np.int64)
    Cb = np.zeros(NBANK, np.int64)
    for j in range(TOTCH):
        bb = b_of_chunk[j]
        q_of_chunk[j] = Cb[bb]
        Cb[bb] += 1

    # per-bank idx streams, wrapped int16 [128, C_b * 8]
    gidx = []
    chunks_src = srcrel_pad.reshape(NCORES, TOTCH, P)
    for bb in range(NBANK):
        sel = chunks_src[:, b_of_chunk == bb, :].reshape(NCORES, -1)
        w = sel.reshape(NCORES, -1, 16).transpose(0, 2, 1)
        gidx.append(np.tile(w, (1, 8, 1)).astype(np.int16))

    dstrel_in = dstrel_pad.reshape(NCORES, TOTCH, P).transpose(0, 2, 1)
    dstrel_in = dstrel_in.astype(ml_dtypes.bfloat16)

    # per-core x feature-major bf16, and per-source dinv
    x = np.asarray(x, dtype=np.float32)
    x_pad = np.zeros((NCORES, NPAD, D), np.float32)
    x_pad[core_of, sidx_of] = x * dinv[:, None]   # fold src-side deg^-1/2
    xcT = np.ascontiguousarray(x_pad.transpose(0, 2, 1)).astype(
        ml_dtypes.bfloat16
    )  # [8, 128, 12800]

    dinv_pad = np.zeros((NCORES, NPAD), np.float32)
    dinv_pad[core_of, sidx_of] = dinv
    dinv_in = np.ascontiguousarray(
        dinv_pad.reshape(NCORES, TILES, P).transpose(0, 2, 1)
    )  # [8, 128, TILES]

    sched = dict(
        K=K, TOTCH=TOTCH, t_of_chunk=t_of_chunk, b_of_chunk=b_of_chunk,
        q_of_chunk=q_of_chunk, Cb=Cb, core_of=core_of, sidx_of=sidx_of,
    )
    return sched, xcT, dinv_in, dstrel_in, gidx


def _build(sched, Cb, triv):
    """triv: dict(b=[bool]*3, g=[bool]*3, bt=[bool]*3) — which params are
    trivial (b==0, g==1, bt==0), letting ops be elided at trace time."""
    from concourse import bass, bacc, mybir, tile
    from concourse.masks import make_identity

    f32 = mybir.dt.float32
    bf16 = mybir.dt.bfloat16
    i16 = mybir.dt.int16

    TOTCH = sched["TOTCH"]
    t_of = sched["t_of_chunk"]
    b_of = sched["b_of_chunk"]
    q_of = sched["q_of_chunk"]

    is_first = np.zeros(TOTCH, bool)
    is_last = np.zeros(TOTCH, bool)
    prev_t = -1
    for j in range(TOTCH):
        if t_of[j] != prev_t:
            is_first[j] = True
            if j > 0:
                is_last[j - 1] = True
            prev_t = t_of[j]
    is_last[TOTCH - 1] = True
    # tiles with no chunks at all (pad tiles)
    tiles_with_chunks = sorted(set(int(t) for t in t_of))

    nc = bacc.Bacc("TRN2", debug=False, num_devices=NCORES, num_swdge_queues=4)

    xcT_d = nc.dram_tensor("xcT", [P, NPAD], bf16, kind="ExternalInput")
    dinv_d = nc.dram_tensor("dinv", [P, TILES], f32, kind="ExternalInput")
    dstrel_d = nc.dram_tensor("dstrel", [P, TOTCH], bf16, kind="ExternalInput")
    gidx_d = [
        nc.dram_tensor(f"gidx{bb}", [P, int(Cb[bb]) * 8], i16, kind="ExternalInput")
        for bb in range(NBANK)
    ]
    w_d = [nc.dram_tensor(f"w{l}", [P, D], f32, kind="ExternalInput") for l in range(3)]
    brep_d = [nc.dram_tensor(f"brep{l}", [P, D], f32, kind="ExternalInput")
              for l in range(3)]
    grep_d = [nc.dram_tensor(f"grep{l}", [P, D], f32, kind="ExternalInput")
              for l in range(3)]
    btrep_d = [nc.dram_tensor(f"btrep{l}", [P, D], f32, kind="ExternalInput")
               for l in range(3)]
    iota_d = nc.dram_tensor("iota", [P, P], bf16, kind="ExternalInput")
    out_d = nc.dram_tensor("out", [NPAD, D], f32, kind="ExternalOutput")

    with tile.TileContext(nc) as tc:
        with (
            tc.tile_pool(name="singles", bufs=1) as singles,
            tc.tile_pool(name="gpool", bufs=GBUFS) as gpool,
            tc.tile_pool(name="spool", bufs=3) as spool,
            tc.tile_pool(name="hstg", bufs=3) as hstg,
            tc.tile_pool(name="ln", bufs=3) as lnp,
            tc.tile_pool(name="psacc", bufs=4, space="PSUM") as psacc,
            tc.tile_pool(name="psmm", bufs=2, space="PSUM") as psmm,
            tc.tile_pool(name="pstp", bufs=2, space="PSUM") as pstp,
            tc.tile_pool(name="dram", bufs=1, space="DRAM") as dram,
        ):
            # ---- persistent SBUF state ----
            xcT = singles.tile([P, NPAD], bf16)
            nc.sync.dma_start(out=xcT[:], in_=xcT_d[:])
            dinv_t = singles.tile([P, TILES], f32)
            nc.sync.dma_start(out=dinv_t[:], in_=dinv_d[:])
            dstrel_t = singles.tile([P, TOTCH], bf16)
            nc.sync.dma_start(out=dstrel_t[:], in_=dstrel_d[:])

            w_t, brep_t, grep_t, btrep_t = [], [], [], []
            for l in range(3):
                wt = singles.tile([P, D], f32, name=f"w{l}")
                nc.sync.dma_start(out=wt[:], in_=w_d[l][:])
                w_t.append(wt)
                if not triv["b"][l]:
                    bt = singles.tile([P, D], f32, name=f"brep{l}")
                    nc.sync.dma_start(out=bt[:], in_=brep_d[l][:])
                    brep_t.append(bt)
                else:
                    brep_t.append(None)
                if not triv["g"][l]:
                    gt = singles.tile([P, D], f32, name=f"grep{l}")
                    nc.sync.dma_start(out=gt[:], in_=grep_d[l][:])
                    grep_t.append(gt)
                else:
                    grep_t.append(None)
                if not triv["bt"][l]:
                    btt = singles.tile([P, D], f32, name=f"btrep{l}")
                    nc.sync.dma_start(out=btt[:], in_=btrep_d[l][:])
                    btrep_t.append(btt)
                else:
                    btrep_t.append(None)
            idx_t = []
            for bb in range(NBANK):
                it0 = singles.tile([P, int(Cb[bb]) * 8], i16, name=f"idxr{bb}")
                nc.sync.dma_start(out=it0[:], in_=gidx_d[bb][:])
                idx_t.append(it0)
            iota_t = singles.tile([P, P], bf16)
            nc.sync.dma_start(out=iota_t[:], in_=iota_d[:])
            ident = singles.tile([P, P], f32)
            make_identity(nc, ident[:])
            eps_t = singles.tile([P, 1], f32)
            nc.vector.memset(eps_t[:], EPS)

            # bf16 weights for fast PE (cast once)
            wb_t = []
            for l in range(3):
                wb = singles.tile([P, D], bf16, name=f"wb{l}")
                nc.vector.tensor_copy(out=wb[:], in_=w_t[l][:])
                wb_t.append(wb)

            # per-layer AG input (own quarter) and gathered table, per quarter
            agin_d = [[dram.tile([QROWS, D], bf16, name=f"agin{l}_{q}")
                       for q in range(QT)] for l in range(3)]
            hfull_d = [[dram.tile([BANKROWS, D], bf16, addr_space="Shared",
                                  name=f"hfull{l}_{q}") for q in range(QT)]
                       for l in range(3)]

            HB = 5  # tiles per staging batch (25 % 5 == 0)

            def stage_tile(l, t, src_ap):
                """Scale rows of tile t by dinv_src, cast bf16, stage; DMA per
                HB-tile batch into agin_d[l][quarter]. src_ap: [P, D] f32/PSUM."""
                q, tq = divmod(t, QTILES)
                if tq % HB == 0:
                    stage_tile.buf = hstg.tile([P, HB, D], bf16, tag="hstage",
                                               name=f"hs{l}_{t}")
                nc.vector.tensor_copy(
                    out=stage_tile.buf[:, tq % HB, :], in_=src_ap)
                if tq % HB == HB - 1:
                    t0 = (tq // HB) * HB
                    nc.sync.dma_start(
                        out=agin_d[l][q][t0 * P:(t0 + HB) * P, :].rearrange(
                            "(c p) d -> p c d", p=P),
                        in_=stage_tile.buf[:],
                    )
                    if tq == QTILES - 1:
                        nc.gpsimd.collective_compute(
                            "AllGather",
                            mybir.AluOpType.bypass,
                            replica_groups=[list(range(NCORES))],
                            ins=[agin_d[l][q].opt()],
                            outs=[hfull_d[l][q].opt()],
                        )

            # ---- layer 0 phase A: h = x@W0 scaled, staged, quarter-AGs ----
            for t in range(TILES):
                hps = psmm.tile([P, D], f32, space="PSUM", tag="hps")
                nc.tensor.matmul(
                    out=hps[:],
                    lhsT=xcT[:, t * P:(t + 1) * P],
                    rhs=wb_t[0][:],
                    start=True, stop=True,
                )
                stage_tile(0, t, hps[:])

            # ---- layers: edge aggregation + per-tile tails ----
            for l in range(NLAYERS):
                gtiles = {}
                gq = 0
                stile = None
                acc = None

                def issue_gather(l, bb, grp):
                    nonlocal gq
                    ng = min(GATHER_GROUP, int(Cb[bb]) - grp * GATHER_GROUP)
                    gt = gpool.tile([P, GATHER_GROUP, P], bf16, tag="gbuf",
                                    name=f"g{l}_{bb}_{grp}")
                    nc.gpsimd.dma_gather(
                        out_ap=gt[:, :ng, :],
                        in_ap=hfull_d[l][bb][:],
                        idxs_ap=idx_t[bb][:, grp * GATHER_GROUP * 8:
                                          (grp * GATHER_GROUP + ng) * 8],
                        num_idxs=ng * P,
                        num_idxs_reg=ng * P,
                        elem_size=P,
                        single_packet=False,
                        queue_num=gq % 4,
                    )
                    gq += 1
                    gtiles[(bb, grp)] = gt

                # Prefetch a runway of bank 0-2 gathers so the first bank-3
                # gather (whose AG lands last) doesn't stall the in-order
                # Pool queue at the layer boundary.
                for grp in range(2):
                    for bb in range(NBANK - 1):
                        if grp * GATHER_GROUP < int(Cb[bb]):
                            issue_gather(l, bb, grp)

                for j in range(TOTCH):
                    t, bb, q = int(t_of[j]), int(b_of[j]), int(q_of[j])
                    grp, slot = divmod(q, GATHER_GROUP)
                    gk = (bb, grp)
                    if gk not in gtiles:
                        issue_gather(l, bb, grp)
                    if j % S_BATCH == 0:
                        nb = min(S_BATCH, TOTCH - j)
                        stile = spool.tile([P, S_BATCH, P], bf16, tag="s",
                                           name=f"s{l}_{j}")
                        nc.vector.tensor_tensor(
                            out=stile[:, :nb, :],
                            in0=iota_t[:, None, :].to_broadcast([P, nb, P]),
                            in1=dstrel_t[:, j:j + nb].to_broadcast([P, nb, P]),
                            op=mybir.AluOpType.is_equal,
                        )
                    if is_first[j]:
                        acc = psacc.tile([P, D], f32, space="PSUM", tag="acc",
                                         name=f"acc{l}_{t}")
                        # seed with this tile's self-messages (scaled x@W)
                        nc.tensor.matmul(
                            out=acc[:],
                            lhsT=xcT[:, t * P:(t + 1) * P],
                            rhs=wb_t[l][:],
                            start=True, stop=False,
                        )
                    nc.tensor.matmul(
                        out=acc[:],
                        lhsT=stile[:, j % S_BATCH, :],
                        rhs=gtiles[gk][:, slot, :],
                        start=False,
                        stop=bool(is_last[j]),
                    )
                    if not is_last[j]:
                        continue

                    # ---- per-tile tail: LN (+ReLU), next-layer A or output ----
                    if triv["b"][l]:
                        conv = acc  # LN(dinv*acc + 0) == LN(acc)
                    else:
                        conv = lnp.tile([P, D], f32, tag="conv")
                        nc.vector.scalar_tensor_tensor(
                            out=conv[:], in0=acc[:],
                            scalar=dinv_t[:, t:t + 1],
                            in1=brep_t[l][:],
                            op0=mybir.AluOpType.mult,
                            op1=mybir.AluOpType.add,
                        )
                    stats = lnp.tile([P, 6], f32, tag="stats")
                    nc.vector.bn_stats(out=stats[:], in_=conv[:])
                    mv = lnp.tile([P, 2], f32, tag="mv")
                    nc.vector.bn_aggr(out=mv[:], in_=stats[:])
                    rstd = lnp.tile([P, 1], f32, tag="rstd")
                    nc.scalar.activation(
                        out=rstd[:], in_=mv[:, 1:2],
                        func=mybir.ActivationFunctionType.Sqrt,
                        bias=eps_t[:],
                    )
                    nc.vector.reciprocal(out=rstd[:], in_=rstd[:])
                    y = lnp.tile([P, D], f32, tag="y")
                    nc.vector.scalar_tensor_tensor(
                        out=y[:], in0=conv[:], scalar=mv[:, 0:1],
                        in1=rstd[:].to_broadcast([P, D]),
                        op0=mybir.AluOpType.subtract,
                        op1=mybir.AluOpType.mult,
                    )
                    if not triv["g"][l]:
                        nc.vector.tensor_mul(out=y[:], in0=y[:], in1=grep_t[l][:])
                    if not triv["bt"][l]:
                        nc.vector.tensor_add(out=y[:], in0=y[:], in1=btrep_t[l][:])

                    if l == NLAYERS - 1:
                        nc.sync.dma_start(
                            out=out_d[t * P:(t + 1) * P, :], in_=y[:])
                        continue
                    nc.scalar.activation(
                        out=y[:], in_=y[:],
                        func=mybir.ActivationFunctionType.Relu,
                        scale=dinv_t[:, t:t + 1],
                    )
                    tp = pstp.tile([P, P], f32, space="PSUM", tag="tp")
                    nc.tensor.transpose(out=tp[:], in_=y[:], identity=ident[:])
                    nc.scalar.copy(out=xcT[:, t * P:(t + 1) * P], in_=tp[:])
                    # next layer phase A for this tile
                    hps = psmm.tile([P, D], f32, space="PSUM", tag="hps")
                    nc.tensor.matmul(
                        out=hps[:],
                        lhsT=xcT[:, t * P:(t + 1) * P],
                        rhs=wb_t[l + 1][:],
                        start=True, stop=True,
                    )
                    stage_tile(l + 1, t, hps[:])

                if l < NLAYERS - 1:
                    # pad tiles (no chunks): their xcT stays zero; stage zeros
                    for t in range(TILES):
                        if t in tiles_with_chunks:
                            continue
                        hps = psmm.tile([P, D], f32, space="PSUM", tag="hps")
                        nc.tensor.matmul(
                            out=hps[:],
                            lhsT=xcT[:, t * P:(t + 1) * P],
                            rhs=wb_t[l + 1][:],
                            start=True, stop=True,
                        )
                        stage_tile(l + 1, t, hps[:])

    nc.compile()
    return nc


def _ensure_ntff_hook():
    import types

    try:
        from antenv.axon_hooks import get_axon_ntff_profile_hook  # noqa: F401
        return
    except ImportError:
        pass
    import antenv

    mod = types.ModuleType("antenv.axon_hooks")
    mod._hook = None

    def set_axon_ntff_profile_hook(h):
        mod._hook = h

    def get_axon_ntff_profile_hook():
        return mod._hook

    mod.set_axon_ntff_profile_hook = set_axon_ntff_profile_hook
    mod.get_axon_ntff_profile_hook = get_axon_ntff_profile_hook
    sys.modules["antenv.axon_hooks"] = mod
    antenv.axon_hooks = mod
    try:
        from trn_agent_boot.trn_boot import _ntff_profile_via_ctypes

        mod._hook = _ntff_profile_via_ctypes("/opt/axon/libaxon_pjrt.so")
    except Exception as e:
        print("ntff hook setup failed:", e)


def kernel(**inputs) -> np.ndarray:
    x = np.asarray(inputs["x"], np.float32)
    edge_index = np.asarray(inputs["edge_index"])
    Ws = [np.asarray(inputs[f"W{l}"], np.float32) for l in range(3)]
    bs = [np.asarray(inputs[f"b{l}"], np.float32) for l in range(3)]
    gs = [np.asarray(inputs[f"g{l}"], np.float32) for l in range(3)]
    bts = [np.asarray(inputs[f"bt{l}"], np.float32) for l in range(3)]

    triv = dict(
        b=[bool(np.all(b == 0)) for b in bs],
        g=[bool(np.all(g == 1)) for g in gs],
        bt=[bool(np.all(bt == 0)) for bt in bts],
    )

    sched, xcT, dinv_in, dstrel_in, gidx = _preprocess(x, edge_index)
    nc = _build(sched, sched["Cb"], triv)

    iota = np.broadcast_to(
        np.arange(P, dtype=np.float32), (P, P)
    ).astype(ml_dtypes.bfloat16)

    in_maps = []
    for c in range(NCORES):
        m = dict(
            xcT=np.ascontiguousarray(xcT[c]),
            dinv=np.ascontiguousarray(dinv_in[c]),
            dstrel=np.ascontiguousarray(dstrel_in[c]),
            iota=np.ascontiguousarray(iota),
        )
        for bb in range(NBANK):
            m[f"gidx{bb}"] = np.ascontiguousarray(gidx[bb][c])
        for l in range(3):
            m[f"w{l}"] = Ws[l]
            m[f"brep{l}"] = np.ascontiguousarray(
                np.broadcast_to(bs[l], (P, D)).astype(np.float32))
            m[f"grep{l}"] = np.ascontiguousarray(
                np.broadcast_to(gs[l], (P, D)).astype(np.float32))
            m[f"btrep{l}"] = np.ascontiguousarray(
                np.broadcast_to(bts[l], (P, D)).astype(np.float32))
        in_maps.append(m)

    from concourse.bass_utils import run_bass_kernel_spmd

    trace = bool(int(os.environ.get("GCN_TRACE", "0")))
    if trace:
        _ensure_ntff_hook()
    res = run_bass_kernel_spmd(
        nc, in_maps, core_ids=list(range(NCORES)), trace=trace
    )
    kernel.last_results = res

    out = np.zeros((N, D), np.float32)
    core_of = sched["core_of"]
    sidx_of = sched["sidx_of"]
    for c in range(NCORES):
        mask = core_of == c
        out[mask] = res.results[c]["out"][sidx_of[mask]]
    return out


# revision 8
# speedup vs baseline: 1.1313x; 1.0734x over previous
"""3-layer GCN encoder on 8 TRN2 NeuronCores — v2.

Strategy (dst-partitioned graph parallel, pipelined):
  - Nodes partitioned across 8 cores (12500 each, padded to NPAD=12800,
    TILES=100 tiles of 128).
  - Per layer, each core holds the full scaled source table
    hfull [8*NPAD, 128] bf16, built by 4 quarter-AllGathers (each quarter =
    25 tiles = 3200 rows per core -> bank of 8*3200=25600 rows,
    int16-addressable).
  - Edge phase: edges grouped by (dst_tile, src_bank); 128-edge chunks
    gathered via dma_gather (4 SWDGE queues) and scatter-added into the dst
    tile's PSUM via one-hot matmuls (S built with is_equal in batches).
  - Per-tile tail: LayerNorm directly on the PSUM accumulator (the dst-side
    deg^-1/2 scale and bias b fold away via LN affine invariance when b==0;
    general path emitted if b!=0), then ReLU, then transpose (PE) and the
    NEXT layer's x@W matmul + dinv_src scale + bf16 staging, so phase A of
    layer l+1 is hidden inside layer l's edge phase. Quarter-AllGathers for
    layer l+1 fire as soon as their 25 tiles are staged.
  - Layer 2's tail writes the output slice instead.

kernel(**inputs) takes FULL inputs, returns the FULL [100000, 128] f32 output.
"""
import os
import sys

sys.path.insert(0, "/opt/trn_rl_repo")

import numpy as np
import ml_dtypes

N = 100000
D = 128
NCORES = 8
P = 128
TILES = 100
NPAD = TILES * P          # 12800
QT = 4                    # quarters (AllGather splits)
QTILES = TILES // QT      # 25 tiles per quarter
QROWS = QTILES * P        # 3200 rows per core per quarter
BANKROWS = NCORES * QROWS # 25600 rows per bank (< 32767)
NBANK = QT
EPS = 1e-5

GATHER_GROUP = int(os.environ.get("GCN_G", "32"))   # chunks per dma_gather
S_BATCH = int(os.environ.get("GCN_SB", "16"))       # chunks per is_equal
GBUFS = int(os.environ.get("GCN_GBUFS", "8"))
NLAYERS = 3


def _preprocess(x, edge_index):
    """Host-side graph preprocessing -> per-core arrays + shared schedule."""
    ei = np.asarray(edge_index)
    # self-loops are NOT materialized as edges: each tile's self-messages are
    # seeded into PSUM by a matmul over the dinv-scaled xcT tile instead.
    src = ei[0].astype(np.int64)
    dst = ei[1].astype(np.int64)
    M = src.shape[0]

    deg = (np.bincount(dst, minlength=N) + 1).astype(np.float32)  # + self-loop
    dinv = 1.0 / np.sqrt(deg)

    # Two-level balanced permutation. The mean (tile, bank) group size sits
    # just under the K=4 chunk boundary (512 edges), so banks are shaped
    # asymmetrically: quarter 0 (whose AllGather lands first each layer)
    # takes a high-out-degree node window summing to ~485K edges and runs
    # its groups at K=5 (<=639); quarters 1-3 are equalized at ~372K so
    # their groups pack under 512 (K=4). Within each quarter, nodes are
    # assigned to (core, tile) bins by a greedy that balances the 4-vector
    # of in-degrees-by-source-quarter with a hard penalty at the chunk
    # boundary, making max-over-cores ~= mean.
    outdeg = np.bincount(src, minlength=N).astype(np.int64)
    order = np.argsort(-outdeg, kind="stable")
    cums = np.concatenate([[0], np.cumsum(outdeg[order])])
    n0 = QTILES * NCORES * P               # 25600 positions in quarter 0
    S0_TARGET = 485500
    r = 0
    while r + n0 <= N and cums[r + n0] - cums[r] > S0_TARGET:
        r += 50
    qassign = np.full(N, -1, np.int64)
    qassign[order[r:r + n0]] = 0
    rest = np.concatenate([order[:r], order[r + n0:]])
    qsizes = [n0, n0, n0, 23 * NCORES * P]   # slots 75-97 in quarter 3
    Sq = np.zeros(3)
    caps_q = np.array([qsizes[1], qsizes[2], qsizes[3]], np.int64)
    for i in rest:
        c = int(np.where(caps_q > 0, -Sq, -np.inf).argmax())
        qassign[i] = c + 1
        Sq[c] += outdeg[i]
        caps_q[c] -= 1

    nvec = np.zeros((N, NBANK), np.int32)     # in-degree by source quarter
    np.add.at(nvec, (dst, qassign[src]), 1)
    tot_in = nvec.sum(axis=1)

    slots_of_q = [list(range(0, 25)), list(range(25, 50)),
                  list(range(50, 75)), list(range(75, 98))]
    TGT = np.array([639.0, 511.0, 511.0, 511.0])
    core_of = np.empty(N, np.int64)
    slot_of = np.empty(N, np.int64)
    pos_of = np.empty(N, np.int64)
    for q in range(QT):
        nodes = np.where(qassign == q)[0]
        nodes = nodes[np.argsort(-tot_in[nodes], kind="stable")]
        bins = [(c, sl) for sl in slots_of_q[q] for c in range(NCORES)]
        nb = len(bins)
        Sb = np.zeros((nb, NBANK), np.float64)
        caps_b = np.full(nb, P, np.int64)
        filled = np.zeros(nb, np.int64)
        V = nvec[nodes].astype(np.float64)
        for k in range(len(nodes)):
            v = V[k]
            Sv = Sb + v
            over = np.maximum(0.0, Sv - (TGT - 0.5))
            cost = (over * over).sum(axis=1) * 1e5 + (Sv * Sv).sum(axis=1)
            cost[caps_b == 0] = np.inf
            bsel = int(np.argmin(cost))
            c, sl = bins[bsel]
            i = nodes[k]
            core_of[i] = c
            slot_of[i] = sl
            pos_of[i] = filled[bsel]
            filled[bsel] += 1
            caps_b[bsel] -= 1
            Sb[bsel] += v
    sidx_of = slot_of * P + pos_of          # row within the core's slice

    # bank = quarter of the source's slice; brel = core*QROWS + row-in-quarter
    q_of_node = sidx_of // QROWS
    brel_of_node = core_of * QROWS + (sidx_of % QROWS)

    core = core_of[dst]
    t = slot_of[dst]
    drel = pos_of[dst]
    b = q_of_node[src]
    srel = brel_of_node[src]

    key = (core * TILES + t) * NBANK + b
    # sort by group, then by source row within group (HBM locality)
    order = np.lexsort((srel, key))
    key_s = key[order]
    core_s = core[order]
    srel_s = srel[order]
    drel_s = drel[order]

    cnt = np.bincount(key, minlength=NCORES * TILES * NBANK).reshape(
        NCORES, TILES, NBANK
    )
    K = np.ceil(cnt.max(axis=0) / P).astype(np.int64)  # [TILES, NBANK]
    Ltb = (K * P).reshape(-1)
    off2 = np.concatenate([[0], np.cumsum(Ltb)[:-1]])
    TOT = int(Ltb.sum())
    TOTCH = TOT // P

    first = np.searchsorted(key_s, key_s, side="left")
    rank = np.arange(M) - first
    pos = off2[(key_s % (TILES * NBANK))] + rank

    srcrel_pad = np.full((NCORES, TOT), -1, np.int64)
    dstrel_pad = np.full((NCORES, TOT), -1.0, np.float32)
    srcrel_pad[core_s, pos] = srel_s
    dstrel_pad[core_s, pos] = drel_s.astype(np.float32)
    # pad slots: duplicate the previous real index (page locality, no garbage)
    for c in range(NCORES):
        row = srcrel_pad[c]
        bad = row < 0
        if bad.any():
            idxs = np.where(~bad, np.arange(TOT), 0)
            np.maximum.accumulate(idxs, out=idxs)
            row[:] = row[idxs]
            row[row < 0] = 0
    srcrel_pad = srcrel_pad.astype(np.int16)

    # chunk schedule: chunk j -> (t, b), bank stream position q
    tb_of_chunk = np.repeat(np.arange(TILES * NBANK), K.reshape(-1))
    t_of_chunk = tb_of_chunk // NBANK
    b_of_chunk = tb_of_chunk % NBANK
    q_of_chunk = np.zeros(TOTCH, np.int64)
    Cb = np.zeros(NBANK, np.int64)
    for j in range(TOTCH):
        bb = b_of_chunk[j]
        q_of_chunk[j] = Cb[bb]
        Cb[bb] += 1

    # per-bank idx streams, wrapped int16 [128, C_b * 8]
    gidx = []
    chunks_src = srcrel_pad.reshape(NCORES, TOTCH, P)
    for bb in range(NBANK):
        sel = chunks_src[:, b_of_chunk == bb, :].reshape(NCORES, -1)
        w = sel.reshape(NCORES, -1, 16).transpose(0, 2, 1)
        gidx.append(np.tile(w, (1, 8, 1)).astype(np.int16))

    dstrel_in = dstrel_pad.reshape(NCORES, TOTCH, P).transpose(0, 2, 1)
    dstrel_in = dstrel_in.astype(ml_dtypes.bfloat16)

    # per-core x feature-major bf16, and per-source dinv
    x = np.asarray(x, dtype=np.float32)
    x_pad = np.zeros((NCORES, NPAD, D), np.float32)
    x_pad[core_of, sidx_of] = x * dinv[:, None]   # fold src-side deg^-1/2
    xcT = np.ascontiguousarray(x_pad.transpose(0, 2, 1)).astype(
        ml_dtypes.bfloat16
    )  # [8, 128, 12800]

    dinv_pad = np.zeros((NCORES, NPAD), np.float32)
    dinv_pad[core_of, sidx_of] = dinv
    dinv_in = np.ascontiguousarray(
        dinv_pad.reshape(NCORES, TILES, P).transpose(0, 2, 1)
    )  # [8, 128, TILES]

    sched = dict(
        K=K, TOTCH=TOTCH, t_of_chunk=t_of_chunk, b_of_chunk=b_of_chunk,
        q_of_chunk=q_of_chunk, Cb=Cb, core_of=core_of, sidx_of=sidx_of,
    )
    return sched, xcT, dinv_in, dstrel_in, gidx


def _build(sched, Cb, triv):
    """triv: dict(b=[bool]*3, g=[bool]*3, bt=[bool]*3) — which params are
    trivial (b==0, g==1, bt==0), letting ops be elided at trace time."""
    from concourse import bass, bacc, mybir, tile
    from concourse.masks import make_identity

    f32 = mybir.dt.float32
    bf16 = mybir.dt.bfloat16
    i16 = mybir.dt.int16

    TOTCH = sched["TOTCH"]
    t_of = sched["t_of_chunk"]
    b_of = sched["b_of_chunk"]
    q_of = sched["q_of_chunk"]

    is_first = np.zeros(TOTCH, bool)
    is_last = np.zeros(TOTCH, bool)
    prev_t = -1
    for j in range(TOTCH):
        if t_of[j] != prev_t:
            is_first[j] = True
            if j > 0:
                is_last[j - 1] = True
            prev_t = t_of[j]
    is_last[TOTCH - 1] = True
    # tiles with no chunks at all (pad tiles)
    tiles_with_chunks = sorted(set(int(t) for t in t_of))

    nc = bacc.Bacc("TRN2", debug=False, num_devices=NCORES, num_swdge_queues=4)

    xcT_d = nc.dram_tensor("xcT", [P, NPAD], bf16, kind="ExternalInput")
    dinv_d = nc.dram_tensor("dinv", [P, TILES], f32, kind="ExternalInput")
    dstrel_d = nc.dram_tensor("dstrel", [P, TOTCH], bf16, kind="ExternalInput")
    gidx_d = [
        nc.dram_tensor(f"gidx{bb}", [P, int(Cb[bb]) * 8], i16, kind="ExternalInput")
        for bb in range(NBANK)
    ]
    w_d = [nc.dram_tensor(f"w{l}", [P, D], f32, kind="ExternalInput") for l in range(3)]
    brep_d = [nc.dram_tensor(f"brep{l}", [P, D], f32, kind="ExternalInput")
              for l in range(3)]
    grep_d = [nc.dram_tensor(f"grep{l}", [P, D], f32, kind="ExternalInput")
              for l in range(3)]
    btrep_d = [nc.dram_tensor(f"btrep{l}", [P, D], f32, kind="ExternalInput")
               for l in range(3)]
    iota_d = nc.dram_tensor("iota", [P, P], bf16, kind="ExternalInput")
    out_d = nc.dram_tensor("out", [NPAD, D], f32, kind="ExternalOutput")

    with tile.TileContext(nc) as tc:
        with (
            tc.tile_pool(name="singles", bufs=1) as singles,
            tc.tile_pool(name="gpool", bufs=GBUFS) as gpool,
            tc.tile_pool(name="spool", bufs=3) as spool,
            tc.tile_pool(name="hstg", bufs=3) as hstg,
            tc.tile_pool(name="ln", bufs=3) as lnp,
            tc.tile_pool(name="psacc", bufs=4, space="PSUM") as psacc,
            tc.tile_pool(name="psmm", bufs=2, space="PSUM") as psmm,
            tc.tile_pool(name="pstp", bufs=2, space="PSUM") as pstp,
            tc.tile_pool(name="dram", bufs=1, space="DRAM") as dram,
        ):
            # ---- persistent SBUF state ----
            xcT = singles.tile([P, NPAD], bf16)
            nc.sync.dma_start(out=xcT[:], in_=xcT_d[:])
            dinv_t = singles.tile([P, TILES], f32)
            nc.sync.dma_start(out=dinv_t[:], in_=dinv_d[:])
            dstrel_t = singles.tile([P, TOTCH], bf16)
            nc.sync.dma_start(out=dstrel_t[:], in_=dstrel_d[:])

            w_t, brep_t, grep_t, btrep_t = [], [], [], []
            for l in range(3):
                wt = singles.tile([P, D], f32, name=f"w{l}")
                nc.sync.dma_start(out=wt[:], in_=w_d[l][:])
                w_t.append(wt)
                if not triv["b"][l]:
                    bt = singles.tile([P, D], f32, name=f"brep{l}")
                    nc.sync.dma_start(out=bt[:], in_=brep_d[l][:])
                    brep_t.append(bt)
                else:
                    brep_t.append(None)
                if not triv["g"][l]:
                    gt = singles.tile([P, D], f32, name=f"grep{l}")
                    nc.sync.dma_start(out=gt[:], in_=grep_d[l][:])
                    grep_t.append(gt)
                else:
                    grep_t.append(None)
                if not triv["bt"][l]:
                    btt = singles.tile([P, D], f32, name=f"btrep{l}")
                    nc.sync.dma_start(out=btt[:], in_=btrep_d[l][:])
                    btrep_t.append(btt)
                else:
                    btrep_t.append(None)
            idx_t = []
            for bb in range(NBANK):
                it0 = singles.tile([P, int(Cb[bb]) * 8], i16, name=f"idxr{bb}")
                nc.sync.dma_start(out=it0[:], in_=gidx_d[bb][:])
                idx_t.append(it0)
            iota_t = singles.tile([P, P], bf16)
            nc.sync.dma_start(out=iota_t[:], in_=iota_d[:])
            ident = singles.tile([P, P], f32)
            make_identity(nc, ident[:])
            eps_t = singles.tile([P, 1], f32)
            nc.vector.memset(eps_t[:], EPS)

            # bf16 weights for fast PE (cast once)
            wb_t = []
            for l in range(3):
                wb = singles.tile([P, D], bf16, name=f"wb{l}")
                nc.scalar.copy(out=wb[:], in_=w_t[l][:])
                wb_t.append(wb)

            # per-layer AG input (own quarter) and gathered table, per quarter
            agin_d = [[dram.tile([QROWS, D], bf16, name=f"agin{l}_{q}")
                       for q in range(QT)] for l in range(3)]
            hfull_d = [[dram.tile([BANKROWS, D], bf16, addr_space="Shared",
                                  name=f"hfull{l}_{q}") for q in range(QT)]
                       for l in range(3)]

            HB = 5  # tiles per staging batch (25 % 5 == 0)

            def stage_tile(l, t, src_ap):
                """Scale rows of tile t by dinv_src, cast bf16, stage; DMA per
                HB-tile batch into agin_d[l][quarter]. src_ap: [P, D] f32/PSUM."""
                q, tq = divmod(t, QTILES)
                if tq % HB == 0:
                    stage_tile.buf = hstg.tile([P, HB, D], bf16, tag="hstage",
                                               name=f"hs{l}_{t}")
                # scalar engine: DVE cast-copies can enter 2-port mode and
                # block GpSimd SWDGE descriptor generation; ACT never does.
                nc.scalar.copy(
                    out=stage_tile.buf[:, tq % HB, :], in_=src_ap)
                if tq % HB == HB - 1:
                    t0 = (tq // HB) * HB
                    nc.sync.dma_start(
                        out=agin_d[l][q][t0 * P:(t0 + HB) * P, :].rearrange(
                            "(c p) d -> p c d", p=P),
                        in_=stage_tile.buf[:],
                    )
                    if tq == QTILES - 1:
                        nc.gpsimd.collective_compute(
                            "AllGather",
                            mybir.AluOpType.bypass,
                            replica_groups=[list(range(NCORES))],
                            ins=[agin_d[l][q].opt()],
                            outs=[hfull_d[l][q].opt()],
                        )

            # ---- layer 0 phase A: h = x@W0 scaled, staged, quarter-AGs ----
            for t in range(TILES):
                hps = psmm.tile([P, D], f32, space="PSUM", tag="hps")
                nc.tensor.matmul(
                    out=hps[:],
                    lhsT=xcT[:, t * P:(t + 1) * P],
                    rhs=wb_t[0][:],
                    start=True, stop=True,
                )
                stage_tile(0, t, hps[:])

            # ---- layers: edge aggregation + per-tile tails ----
            for l in range(NLAYERS):
                gtiles = {}
                gq = 0
                stile = None
                acc = None

                def issue_gather(l, bb, grp):
                    nonlocal gq
                    ng = min(GATHER_GROUP, int(Cb[bb]) - grp * GATHER_GROUP)
                    gt = gpool.tile([P, GATHER_GROUP, P], bf16, tag="gbuf",
                                    name=f"g{l}_{bb}_{grp}")
                    nc.gpsimd.dma_gather(
                        out_ap=gt[:, :ng, :],
                        in_ap=hfull_d[l][bb][:],
                        idxs_ap=idx_t[bb][:, grp * GATHER_GROUP * 8:
                                          (grp * GATHER_GROUP + ng) * 8],
                        num_idxs=ng * P,
                        num_idxs_reg=ng * P,
                        elem_size=P,
                        single_packet=False,
                        queue_num=gq % 4,
                    )
                    gq += 1
                    gtiles[(bb, grp)] = gt

                # Prefetch a runway of bank 0-2 gathers so the first bank-3
                # gather (whose AG lands last) doesn't stall the in-order
                # Pool queue at the layer boundary.
                for grp in range(2):
                    for bb in range(NBANK - 1):
                        if grp * GATHER_GROUP < int(Cb[bb]):
                            issue_gather(l, bb, grp)

                for j in range(TOTCH):
                    t, bb, q = int(t_of[j]), int(b_of[j]), int(q_of[j])
                    grp, slot = divmod(q, GATHER_GROUP)
                    gk = (bb, grp)
                    if gk not in gtiles:
                        issue_gather(l, bb, grp)
                    if j % S_BATCH == 0:
                        nb = min(S_BATCH, TOTCH - j)
                        stile = spool.tile([P, S_BATCH, P], bf16, tag="s",
                                           name=f"s{l}_{j}")
                        nc.vector.tensor_tensor(
                            out=stile[:, :nb, :],
                            in0=iota_t[:, None, :].to_broadcast([P, nb, P]),
                            in1=dstrel_t[:, j:j + nb].to_broadcast([P, nb, P]),
                            op=mybir.AluOpType.is_equal,
                        )
                    if is_first[j]:
                        acc = psacc.tile([P, D], f32, space="PSUM", tag="acc",
                                         name=f"acc{l}_{t}")
                        # seed with this tile's self-messages (scaled x@W)
                        nc.tensor.matmul(
                            out=acc[:],
                            lhsT=xcT[:, t * P:(t + 1) * P],
                            rhs=wb_t[l][:],
                            start=True, stop=False,
                        )
                    nc.tensor.matmul(
                        out=acc[:],
                        lhsT=stile[:, j % S_BATCH, :],
                        rhs=gtiles[gk][:, slot, :],
                        start=False,
                        stop=bool(is_last[j]),
                    )
                    if not is_last[j]:
                        continue

                    # ---- per-tile tail: LN (+ReLU), next-layer A or output ----
                    if triv["b"][l]:
                        conv = acc  # LN(dinv*acc + 0) == LN(acc)
                    else:
                        conv = lnp.tile([P, D], f32, tag="conv")
                        nc.vector.scalar_tensor_tensor(
                            out=conv[:], in0=acc[:],
                            scalar=dinv_t[:, t:t + 1],
                            in1=brep_t[l][:],
                            op0=mybir.AluOpType.mult,
                            op1=mybir.AluOpType.add,
                        )
                    stats = lnp.tile([P, 6], f32, tag="stats")
                    nc.vector.bn_stats(out=stats[:], in_=conv[:])
                    mv = lnp.tile([P, 2], f32, tag="mv")
                    nc.vector.bn_aggr(out=mv[:], in_=stats[:])
                    rstd = lnp.tile([P, 1], f32, tag="rstd")
                    nc.scalar.activation(
                        out=rstd[:], in_=mv[:, 1:2],
                        func=mybir.ActivationFunctionType.Sqrt,
                        bias=eps_t[:],
                    )
                    nc.vector.reciprocal(out=rstd[:], in_=rstd[:])
                    y = lnp.tile([P, D], f32, tag="y")
                    nc.vector.scalar_tensor_tensor(
                        out=y[:], in0=conv[:], scalar=mv[:, 0:1],
                        in1=rstd[:].to_broadcast([P, D]),
                        op0=mybir.AluOpType.subtract,
                        op1=mybir.AluOpType.mult,
                    )
                    if not triv["g"][l]:
                        nc.vector.tensor_mul(out=y[:], in0=y[:], in1=grep_t[l][:])
                    if not triv["bt"][l]:
                        nc.vector.tensor_add(out=y[:], in0=y[:], in1=btrep_t[l][:])

                    if l == NLAYERS - 1:
                        nc.sync.dma_start(
                            out=out_d[t * P:(t + 1) * P, :], in_=y[:])
                        continue
                    nc.scalar.activation(
                        out=y[:], in_=y[:],
                        func=mybir.ActivationFunctionType.Relu,
                        scale=dinv_t[:, t:t + 1],
                    )
                    tp = pstp.tile([P, P], f32, space="PSUM", tag="tp")
                    nc.tensor.transpose(out=tp[:], in_=y[:], identity=ident[:])
                    nc.scalar.copy(out=xcT[:, t * P:(t + 1) * P], in_=tp[:])
                    # next layer phase A for this tile
                    hps = psmm.tile([P, D], f32, space="PSUM", tag="hps")
                    nc.tensor.matmul(
                        out=hps[:],
                        lhsT=xcT[:, t * P:(t + 1) * P],
                        rhs=wb_t[l + 1][:],
                        start=True, stop=True,
                    )
                    stage_tile(l + 1, t, hps[:])

                if l < NLAYERS - 1:
                    # pad tiles (no chunks): their xcT stays zero; stage zeros
                    for t in range(TILES):
                        if t in tiles_with_chunks:
                            continue
                        hps = psmm.tile([P, D], f32, space="PSUM", tag="hps")
                        nc.tensor.matmul(
                            out=hps[:],
                            lhsT=xcT[:, t * P:(t + 1) * P],
                            rhs=wb_t[l + 1][:],
                            start=True, stop=True,
                        )
                        stage_tile(l + 1, t, hps[:])

    nc.compile()
    return nc


def _ensure_ntff_hook():
    import types

    try:
        from antenv.axon_hooks import get_axon_ntff_profile_hook  # noqa: F401
        return
    except ImportError:
        pass
    import antenv

    mod = types.ModuleType("antenv.axon_hooks")
    mod._hook = None

    def set_axon_ntff_profile_hook(h):
        mod._hook = h

    def get_axon_ntff_profile_hook():
        return mod._hook

    mod.set_axon_ntff_profile_hook = set_axon_ntff_profile_hook
    mod.get_axon_ntff_profile_hook = get_axon_ntff_profile_hook
    sys.modules["antenv.axon_hooks"] = mod
    antenv.axon_hooks = mod
    try:
        from trn_agent_boot.trn_boot import _ntff_profile_via_ctypes

        mod._hook = _ntff_profile_via_ctypes("/opt/axon/libaxon_pjrt.so")
    except Exception as e:
        print("ntff hook setup failed:", e)


def kernel(**inputs) -> np.ndarray:
    x = np.asarray(inputs["x"], np.float32)
    edge_index = np.asarray(inputs["edge_index"])
    Ws = [np.asarray(inputs[f"W{l}"], np.float32) for l in range(3)]
    bs = [np.asarray(inputs[f"b{l}"], np.float32) for l in range(3)]
    gs = [np.asarray(inputs[f"g{l}"], np.float32) for l in range(3)]
    bts = [np.asarray(inputs[f"bt{l}"], np.float32) for l in range(3)]

    triv = dict(
        b=[bool(np.all(b == 0)) for b in bs],
        g=[bool(np.all(g == 1)) for g in gs],
        bt=[bool(np.all(bt == 0)) for bt in bts],
    )

    sched, xcT, dinv_in, dstrel_in, gidx = _preprocess(x, edge_index)
    nc = _build(sched, sched["Cb"], triv)

    iota = np.broadcast_to(
        np.arange(P, dtype=np.float32), (P, P)
    ).astype(ml_dtypes.bfloat16)

    in_maps = []
    for c in range(NCORES):
        m = dict(
            xcT=np.ascontiguousarray(xcT[c]),
            dinv=np.ascontiguousarray(dinv_in[c]),
            dstrel=np.ascontiguousarray(dstrel_in[c]),
            iota=np.ascontiguousarray(iota),
        )
        for bb in range(NBANK):
            m[f"gidx{bb}"] = np.ascontiguousarray(gidx[bb][c])
        for l in range(3):
            m[f"w{l}"] = Ws[l]
            m[f"brep{l}"] = np.ascontiguousarray(
                np.broadcast_to(bs[l], (P, D)).astype(np.float32))
            m[f"grep{l}"] = np.ascontiguousarray(
                np.broadcast_to(gs[l], (P, D)).astype(np.float32))
            m[f"btrep{l}"] = np.ascontiguousarray(
                np.broadcast_to(bts[l], (P, D)).astype(np.float32))
        in_maps.append(m)

    from concourse.bass_utils import run_bass_kernel_spmd

    trace = bool(int(os.environ.get("GCN_TRACE", "0")))
    if trace:
        _ensure_ntff_hook()
    res = run_bass_kernel_spmd(
        nc, in_maps, core_ids=list(range(NCORES)), trace=trace
    )
    kernel.last_results = res

    out = np.zeros((N, D), np.float32)
    core_of = sched["core_of"]
    sidx_of = sched["sidx_of"]
    for c in range(NCORES):
        mask = core_of == c
        out[mask] = res.results[c]["out"][sidx_of[mask]]
    return out
